# revision 1
# baseline (speedup 1.0000x reference)
"""Trainium2 Bass kernel for EnhancedGatedFusion (MoE routing, top-2 of 8 experts).

Strategy: data-parallel over tokens across 8 NeuronCores. Each core gets
T=1024 tokens (full weights replicated) and computes:
  router logits (true fp32 matmul - top-2 selection is precision critical),
  top-2 softmax gate weights via dense max/mask trick,
  dense 8-expert MLP (float32r matmuls at full PE rate) accumulated in a
  transposed C^T [D, T] layout so expert bias is per-partition and no
  transpose is needed before the projection matmul,
  projection + residual + RMSNorm in token-major layout.
"""

import sys

for _p in ("/opt/trn_rl_repo",):
    if _p not in sys.path:
        sys.path.insert(0, _p)

from contextlib import ExitStack

import numpy as np

import concourse.bass as bass
import concourse.mybir as mybir
import concourse.tile as tile
from concourse import bacc
from concourse.masks import make_identity

FP32 = mybir.dt.float32
FP32R = mybir.dt.float32r
BF16 = mybir.dt.bfloat16
AX = mybir.AxisListType
ALU = mybir.AluOpType
ACTF = mybir.ActivationFunctionType

EPS = 1e-6
NEG_BIG = -1e30


def _bcast_ap(ap, nparts=128):
    """Partition-broadcast view of a DRAM AP (step-0 partition dim)."""
    return bass.AP(tensor=ap.tensor, offset=ap.offset, ap=[[0, nparts], *ap.ap])


def build_moe_nc(D, E, T, PW=256, trn_type="TRN2", expert_bf16=False):
    """Emit the per-core MoE program. Returns a compiled Bacc instance.

    D: model dim (multiple of 128); E: num experts; T: tokens per core;
    PW: weight panel width (multiple of 128, >=256 for f32r full rate on proj).
    """
    P = 128
    KO = D // P          # contraction k-tiles
    NTT = T // P         # token tiles of 128
    TOKMM = min(512, T)  # moving-operand token chunk for expert matmuls
    NTH = T // TOKMM     # token chunks
    NCP = D // PW        # weight panels (expert cols / proj cols)
    NCT = PW // P        # col-tiles of 128 per panel

    nc = bacc.Bacc(trn_type, target_bir_lowering=False, debug=False)

    xt = nc.dram_tensor("xt", [D, T], FP32, kind="ExternalInput").ap()
    xtb = (nc.dram_tensor("xtb", [D, T], BF16, kind="ExternalInput").ap()
           if expert_bf16 else None)
    xr = nc.dram_tensor("xr", [T, D], FP32, kind="ExternalInput").ap()
    router_w = nc.dram_tensor("router_w", [D, E], FP32, kind="ExternalInput").ap()
    router_b = nc.dram_tensor("router_b", [E], FP32, kind="ExternalInput").ap()
    ew_dt = BF16 if expert_bf16 else FP32
    expert_w = nc.dram_tensor("expert_w", [E, D, D], ew_dt, kind="ExternalInput").ap()
    expert_b = nc.dram_tensor("expert_b", [E, D], FP32, kind="ExternalInput").ap()
    proj_w = nc.dram_tensor("proj_w", [D, D], FP32, kind="ExternalInput").ap()
    proj_b = nc.dram_tensor("proj_b", [D], FP32, kind="ExternalInput").ap()
    norm_w = nc.dram_tensor("norm_w", [D], FP32, kind="ExternalInput").ap()
    out = nc.dram_tensor("out", [T, D], FP32, kind="ExternalOutput").ap()
    fw_dram = nc.dram_tensor("fw_scratch", [E, T], FP32).ap()

    xt_r = xt.rearrange("(ko p) t -> p ko t", p=P)
    rw_r = router_w.rearrange("(ko p) e -> p ko e", p=P)

    with tile.TileContext(nc) as tc, ExitStack() as ctx:
        v = nc.vector
        s = nc.scalar

        big = ctx.enter_context(tc.tile_pool(name="big", bufs=1))
        ct_pool = ctx.enter_context(tc.tile_pool(name="ct_pool", bufs=1))
        w_pool = ctx.enter_context(tc.tile_pool(name="w_pool", bufs=2))
        sil_pool = ctx.enter_context(tc.tile_pool(name="sil_pool", bufs=3))
        small = ctx.enter_context(tc.tile_pool(name="small", bufs=2))
        singles = ctx.enter_context(tc.tile_pool(name="singles", bufs=1))
        xres_pool = ctx.enter_context(tc.tile_pool(name="xres_pool", bufs=1 if expert_bf16 else 2))

        # ---- resident loads (small tensors first so the router's
        # weights aren't queued behind 8MiB of xt traffic) ----
        rw_sb = singles.tile([P, KO, E], FP32)
        nc.sync.dma_start(out=rw_sb, in_=rw_r)
        rb_rep = singles.tile([P, E], FP32)
        nc.sync.dma_start(out=rb_rep, in_=_bcast_ap(router_b))
        nw_rep = singles.tile([P, D], FP32)
        nc.sync.dma_start(out=nw_rep, in_=_bcast_ap(norm_w))
        if expert_bf16:
            xmm_sb = big.tile([P, KO, T], BF16, tag="big", name="xtb_sb")
            xtb_r = xtb.rearrange("(ko p) t -> p ko t", p=P)
            for ko in range(KO):
                nc.sync.dma_start(out=xmm_sb[:, ko, :], in_=xtb_r[:, ko, :])
            rxt_pool = ctx.enter_context(tc.tile_pool(name="rxt_pool", bufs=1))
        else:
            xmm_sb = big.tile([P, KO, T], FP32R, tag="big", name="xt_sb")
            for ko in range(KO):
                eng = nc.sync if ko % 2 == 0 else nc.scalar
                eng.dma_start(
                    out=xmm_sb[:, ko, :], in_=xt_r[:, ko, :].bitcast(FP32R)
                )

        identity = singles.tile([P, P], FP32)
        make_identity(nc, identity)
        eps_t = singles.tile([P, 1], FP32)
        v.memset(eps_t, EPS)

        fwT = singles.tile([E, T], FP32)  # gate weights, expert-major
        ct = ct_pool.tile([P, KO, T], FP32R)  # C^T accumulator [D, T]

        pse = tc.alloc_tile_pool(name="pse", bufs=6, space="PSUM")

        def panel_mms(eidx, cq, wp):
            tiles = []
            for c2 in range(NCT):
                for th in range(NTH):
                    ps = pse.tile([P, TOKMM], FP32, tag="ps",
                                  name=f"ps{eidx}_{cq}_{c2}_{th}")
                    for ko in range(KO):
                        nc.tensor.matmul(
                            ps,
                            lhsT=wp[:, ko, c2 * P:(c2 + 1) * P],
                            rhs=xmm_sb[:, ko, th * TOKMM:(th + 1) * TOKMM],
                            start=(ko == 0),
                            stop=(ko == KO - 1),
                        )
                    tiles.append(ps)
            return tiles

        def panel_gating(eidx, cq, tiles, fw_rep, eb_sb):
            idx = 0
            for c2 in range(NCT):
                colt = cq * NCT + c2
                for th in range(NTH):
                    ps = tiles[idx]
                    idx += 1
                    sg = sil_pool.tile([P, TOKMM], FP32, tag="sg",
                                       name=f"sg{eidx}_{cq}_{c2}_{th}")
                    s.activation(
                        sg, ps, ACTF.Sigmoid, bias=eb_sb[:, colt:colt + 1]
                    )
                    sil = sil_pool.tile([P, TOKMM], FP32, tag="sil",
                                        name=f"sil{eidx}_{cq}_{c2}_{th}")
                    v.scalar_tensor_tensor(
                        out=sil, in0=ps, scalar=eb_sb[:, colt:colt + 1],
                        in1=sg, op0=ALU.add, op1=ALU.mult,
                    )
                    ct_sl = ct[:, colt, th * TOKMM:(th + 1) * TOKMM]
                    fw_sl = fw_rep[:, th * TOKMM:(th + 1) * TOKMM]
                    if eidx == 0:
                        v.tensor_tensor(out=ct_sl, in0=sil, in1=fw_sl,
                                        op=ALU.mult)
                    else:
                        v.tensor_tensor(out=sil, in0=sil, in1=fw_sl,
                                        op=ALU.mult)
                        v.tensor_tensor(out=ct_sl, in0=ct_sl, in1=sil,
                                        op=ALU.add)

        def load_panel(eidx, cq, we_r):
            if expert_bf16:
                wp = w_pool.tile([P, KO, PW], BF16, tag="wp",
                                 name=f"wp{eidx}_{cq}")
                nc.sync.dma_start(out=wp, in_=we_r[:, :, cq * PW:(cq + 1) * PW])
            else:
                wp = w_pool.tile([P, KO, PW], FP32R, tag="wp",
                                 name=f"wp{eidx}_{cq}")
                weng = nc.sync if cq % 2 == 0 else nc.scalar
                weng.dma_start(
                    out=wp, in_=we_r[:, :, cq * PW:(cq + 1) * PW].bitcast(FP32R)
                )
            return wp

        # head start: expert 0's first panel matmuls fill the PE while xt
        # finishes loading and the router's DVE chain runs
        we0_r = expert_w[0].rearrange("(ko p) c -> p ko c", p=P)
        eb0_sb = small.tile([P, KO], FP32, name="eb0")
        nc.scalar.dma_start(
            out=eb0_sb, in_=expert_b[0].rearrange("(ko p) -> p ko", p=P)
        )
        wp00 = load_panel(0, 0, we0_r)
        head_tiles = panel_mms(0, 0, wp00)

        # ---- router + top-2 softmax gates ----
        with (
            tc.tile_pool(name="psr", bufs=1, space="PSUM") as psr,
            tc.tile_pool(name="pst", bufs=1, space="PSUM") as pst,
            tc.tile_pool(name="rsm", bufs=2) as rsm,
            tc.tile_pool(name="fwp", bufs=NTT) as fwp,
        ):
            fw_tiles = []
            for tt in range(NTT):
                if expert_bf16:
                    xtf = rxt_pool.tile([P, KO, P], FP32, tag="rxt")
                    nc.sync.dma_start(
                        out=xtf, in_=xt_r[:, :, tt * P:(tt + 1) * P]
                    )
                else:
                    xtf = xmm_sb[:, :, tt * P:(tt + 1) * P].bitcast(FP32)
                ps_l = psr.tile([P, E], FP32)
                for ko in range(KO):
                    nc.tensor.matmul(
                        ps_l,
                        lhsT=xtf[:, ko, :],
                        rhs=rw_sb[:, ko, :],
                        start=(ko == 0),
                        stop=(ko == KO - 1),
                    )
                logits = rsm.tile([P, E], FP32)
                v.tensor_tensor(out=logits, in0=ps_l, in1=rb_rep, op=ALU.add)
                m1 = rsm.tile([P, 1], FP32)
                v.tensor_reduce(m1, logits, axis=AX.X, op=ALU.max)
                mask1 = rsm.tile([P, E], FP32)
                v.tensor_scalar(mask1, logits, m1, None, op0=ALU.is_ge)
                lg2 = rsm.tile([P, E], FP32)
                v.scalar_tensor_tensor(
                    out=lg2, in0=mask1, scalar=NEG_BIG, in1=logits,
                    op0=ALU.mult, op1=ALU.add,
                )
                m2 = rsm.tile([P, 1], FP32)
                v.tensor_reduce(m2, lg2, axis=AX.X, op=ALU.max)
                mask2 = rsm.tile([P, E], FP32)
                v.tensor_scalar(mask2, lg2, m2, None, op0=ALU.is_ge)
                d21 = rsm.tile([P, 1], FP32)
                v.tensor_tensor(out=d21, in0=m2, in1=m1, op=ALU.subtract)
                e2 = rsm.tile([P, 1], FP32)
                s.activation(e2, d21, ACTF.Exp)
                den = rsm.tile([P, 1], FP32)
                v.tensor_scalar(den, e2, 1.0, None, op0=ALU.add)
                winv = rsm.tile([P, 1], FP32)
                v.reciprocal(winv, den)
                w2 = rsm.tile([P, 1], FP32)
                v.tensor_tensor(out=w2, in0=e2, in1=winv, op=ALU.mult)
                t2 = rsm.tile([P, E], FP32)
                v.tensor_scalar(t2, mask2, w2, None, op0=ALU.mult)
                fw = fwp.tile([P, E], FP32, tag="fw", name=f"fw{tt}")
                v.scalar_tensor_tensor(
                    out=fw, in0=mask1, scalar=winv, in1=t2,
                    op0=ALU.mult, op1=ALU.add,
                )
                fw_tiles.append(fw)
            for tt in range(NTT):
                ps_t = pst.tile([E, P], FP32)
                nc.tensor.transpose(ps_t, fw_tiles[tt], identity)
                v.tensor_copy(out=fwT[:, tt * P:(tt + 1) * P], in_=ps_t)
            nc.sync.dma_start(out=fw_dram, in_=fwT)

        # ---- expert phase: ct[d, t] = sum_e gate[e,t] * silu(x @ We + be)^T ----
        for e in range(E):
            fw_rep = sil_pool.tile([P, T], FP32, tag="fwrep",
                                   bufs=1 if expert_bf16 else 2,
                                   name=f"fwrep{e}")
            nc.sync.dma_start(out=fw_rep, in_=_bcast_ap(fw_dram[e]))
            if e == 0:
                eb_sb = eb0_sb
                we_r = we0_r
            else:
                eb_sb = small.tile([P, KO], FP32, name=f"eb{e}")
                nc.sync.dma_start(
                    out=eb_sb, in_=expert_b[e].rearrange("(ko p) -> p ko", p=P)
                )
                we_r = expert_w[e].rearrange("(ko p) c -> p ko c", p=P)
            for cq in range(NCP):
                if e == 0 and cq == 0:
                    tiles = head_tiles
                else:
                    wp = load_panel(e, cq, we_r)
                    tiles = panel_mms(e, cq, wp)
                panel_gating(e, cq, tiles, fw_rep, eb_sb)

        pse.release()

        # ---- projection + residual into Y (token-major), reusing xt's slot ----
        y_all = big.tile([P, NTT, D], FP32, tag="big")
        pw_r = proj_w.rearrange("(ko p) c -> p ko c", p=P)
        with (
            tc.tile_pool(name="psp", bufs=6, space="PSUM") as psp,
            tc.tile_pool(name="nsm", bufs=2) as nsm,
        ):
            HD = D // 2

            def emit_norm(tt):
                # RMS norm (in place on Y[tt]) + store, interleaved with proj
                y_t = y_all[:, tt, :]
                sq = nsm.tile([P, HD], FP32, tag="sq", bufs=1, name=f"sq{tt}")
                ssa = nsm.tile([P, 1], FP32, tag="ssa", name=f"ssa{tt}")
                ssb = nsm.tile([P, 1], FP32, tag="ssb", name=f"ssb{tt}")
                s.activation(sq, y_t[:, :HD], ACTF.Square, accum_out=ssa)
                s.activation(sq, y_t[:, HD:], ACTF.Square, accum_out=ssb)
                ssum = nsm.tile([P, 1], FP32, tag="ssum", name=f"ssum{tt}")
                v.tensor_tensor(out=ssum, in0=ssa, in1=ssb, op=ALU.add)
                rms = nsm.tile([P, 1], FP32, tag="rms", name=f"rms{tt}")
                s.activation(rms, ssum, ACTF.Sqrt, bias=eps_t, scale=1.0 / D)
                rinv = nsm.tile([P, 1], FP32, tag="rinv", name=f"rinv{tt}")
                v.reciprocal(rinv, rms)
                s.mul(y_t, y_t, rinv)
                v.tensor_tensor(out=y_t, in0=y_t, in1=nw_rep, op=ALU.mult)
                oeng = nc.sync if tt % 2 == 0 else nc.scalar
                oeng.dma_start(out=out[tt * P:(tt + 1) * P, :], in_=y_t)

            NG = min(2, NTT)
            TG = NTT // NG
            for tg, pp in [(g, p) for g in range(NG) for p in range(NCP)]:
                pwp = w_pool.tile([P, KO, PW], FP32R, tag="wp")
                nc.sync.dma_start(out=pwp, in_=pw_r[:, :, pp * PW:(pp + 1) * PW].bitcast(FP32R))
                prb = xres_pool.tile([P, PW], FP32, tag="prb", bufs=2)
                nc.scalar.dma_start(out=prb, in_=_bcast_ap(proj_b[pp * PW:(pp + 1) * PW]))
                for tt in range(tg * TG, (tg + 1) * TG):
                    ps_o = psp.tile([P, PW], FP32)
                    for ko in range(KO):
                        nc.tensor.matmul(
                            ps_o,
                            lhsT=ct[:, ko, tt * P:(tt + 1) * P],
                            rhs=pwp[:, ko, :],
                            start=(ko == 0),
                            stop=(ko == KO - 1),
                        )
                    xres = xres_pool.tile([P, PW], FP32)
                    nc.scalar.dma_start(
                        out=xres,
                        in_=xr[tt * P:(tt + 1) * P, pp * PW:(pp + 1) * PW],
                    )
                    y_sl = y_all[:, tt, pp * PW:(pp + 1) * PW]
                    v.tensor_tensor(out=y_sl, in0=ps_o, in1=prb, op=ALU.add)
                    v.tensor_tensor(out=y_sl, in0=y_sl, in1=xres, op=ALU.add)
                    if pp == NCP - 1:
                        emit_norm(tt)

    nc.compile()
    return nc


# ---- full-problem entry point ----
_B, _S, _D, _E = 4, 2048, 2048, 8
_NCORES = 8
_T = _B * _S // _NCORES

_EXPERT_BF16 = False

_nc_cache = None


def _get_nc():
    global _nc_cache
    if _nc_cache is None:
        _nc_cache = build_moe_nc(_D, _E, _T, expert_bf16=_EXPERT_BF16)
    return _nc_cache


def _make_in_maps(xf, router_w, router_b, expert_w, expert_b, proj_w, proj_b,
                  norm_w):
    if _EXPERT_BF16:
        import ml_dtypes
        expert_w_c = expert_w.astype(ml_dtypes.bfloat16)
    else:
        expert_w_c = expert_w
    in_maps = []
    for c in range(_NCORES):
        xs = xf[c * _T:(c + 1) * _T]
        xst = np.ascontiguousarray(xs.T)
        m = {
            "xt": xst,
            "xr": np.ascontiguousarray(xs),
            "router_w": router_w,
            "router_b": router_b,
            "expert_w": expert_w_c,
            "expert_b": expert_b,
            "proj_w": proj_w,
            "proj_b": proj_b,
            "norm_w": norm_w,
        }
        if _EXPERT_BF16:
            import ml_dtypes
            m["xtb"] = xst.astype(ml_dtypes.bfloat16)
        in_maps.append(m)
    return in_maps


def kernel(x, router_w, router_b, expert_w, expert_b, proj_w, proj_b, norm_w):
    from concourse import bass_utils

    x = np.asarray(x, np.float32)
    router_w = np.asarray(router_w, np.float32)
    router_b = np.asarray(router_b, np.float32)
    expert_w = np.asarray(expert_w, np.float32)
    expert_b = np.asarray(expert_b, np.float32)
    proj_w = np.asarray(proj_w, np.float32)
    proj_b = np.asarray(proj_b, np.float32)
    norm_w = np.asarray(norm_w, np.float32)

    nc = _get_nc()
    xf = x.reshape(-1, _D)
    in_maps = _make_in_maps(xf, router_w, router_b, expert_w, expert_b,
                            proj_w, proj_b, norm_w)
    res = bass_utils.run_bass_kernel_spmd(nc, in_maps, core_ids=list(range(_NCORES)))
    outs = [res.results[c]["out"] for c in range(_NCORES)]
    return np.concatenate(outs, axis=0).reshape(_B, _S, _D).astype(np.float32)



# revision 2
# speedup vs baseline: 1.1273x; 1.1273x over previous
"""Trainium2 Bass kernel for EnhancedGatedFusion (MoE top-2 of 8), v2: sparse dispatch.

Strategy: data-parallel over tokens across 8 cores (T=1024 tokens each).
Per core:
  1. Router in true fp32 (top-2 selection is precision critical): stream
     x^T ko-slices, matmul vs router_w -> logits^T [E, T] in PSUM, add bias,
     transpose to token-major, dense top-2 softmax via max/mask trick.
  2. Build per-expert compact token lists ON DEVICE with no dynamic
     addressing: prefix-sum the routed-mask along tokens (tensor_tensor_scan),
     form one-hot G[t, slot] tiles via iota/is_equal, then tiny matmuls
     G^T @ [t, 1, gates] give, per compact slot: token idx, validity, gate.
     Indices round-trip through DRAM into the [16, n/16] int16 layout the
     SWDGE gather/scatter ops require (replicated across 8 partition groups).
  3. dma_gather(transpose=True) fetches the C=384-padded token set for each
     expert from token-major bf16 x in DRAM, landing feature-major
     [128, KO, C] -- the exact lhsT layout for the expert matmul.
  4. Expert matmul (bf16): x-compact stationary, expert_w streamed in halves;
     silu(h + b) * gate on scalar/vector engines (padding slots have gate 0).
  5. dma_scatter_add accumulates contributions into a DRAM comb buffer
     (fp32); padding slots carry exact zeros and are directed at trash rows
     past the real tokens so their adds cannot race real rows.
  6. comb is read back, converted bf16, PE-transposed, and projected
     (bf16) + bias + residual + RMSNorm as in the dense baseline.
"""

import sys

for _p in ("/opt/trn_rl_repo",):
    if _p not in sys.path:
        sys.path.insert(0, _p)

from contextlib import ExitStack

import numpy as np

import concourse.bass as bass
import concourse.mybir as mybir
import concourse.tile as tile
from concourse import bacc
from concourse.masks import make_identity

FP32 = mybir.dt.float32
FP32R = mybir.dt.float32r
BF16 = mybir.dt.bfloat16
I16 = mybir.dt.int16
I32 = mybir.dt.int32
AX = mybir.AxisListType
ALU = mybir.AluOpType
ACTF = mybir.ActivationFunctionType

EPS = 1e-6
NEG_BIG = -1e30


def _bcast_ap(ap, nparts=128):
    """Partition-broadcast view of a DRAM AP (step-0 partition dim)."""
    return bass.AP(tensor=ap.tensor, offset=ap.offset, ap=[[0, nparts], *ap.ap])


def build_moe_v2(D, E, T, C=384, trn_type="TRN2"):
    """Per-core sparse MoE program. D model dim, E experts, T tokens/core,
    C compact capacity per expert (multiple of 128 slots, >= max expert load)."""
    P = 128
    KO = D // P            # contraction k-tiles
    NTT = T // P           # token tiles
    NCT = C // P           # compact c-tiles per expert
    NIC = C // 16          # idx columns in the [16, C/16] swdge layout
    TRASH = T              # scatter row for padding slots
    TCOMB = ((T + C) + P - 1) // P * P  # comb rows incl. trash, multiple of 128
    NH = 2                 # expert_w streamed in halves
    HW = D // NH           # half width

    nc = bacc.Bacc(trn_type, target_bir_lowering=False, debug=False)

    xt = nc.dram_tensor("xt", [D, T], FP32, kind="ExternalInput").ap()
    xrb = nc.dram_tensor("xrb", [T, D], BF16, kind="ExternalInput").ap()
    xr = nc.dram_tensor("xr", [T, D], FP32, kind="ExternalInput").ap()
    router_w = nc.dram_tensor("router_w", [D, E], FP32, kind="ExternalInput").ap()
    router_b = nc.dram_tensor("router_b", [E], FP32, kind="ExternalInput").ap()
    expert_w = nc.dram_tensor("expert_w", [E, D, D], BF16, kind="ExternalInput").ap()
    expert_b = nc.dram_tensor("expert_b", [E, D], FP32, kind="ExternalInput").ap()
    proj_w = nc.dram_tensor("proj_w", [D, D], BF16, kind="ExternalInput").ap()
    proj_b = nc.dram_tensor("proj_b", [D], FP32, kind="ExternalInput").ap()
    norm_w = nc.dram_tensor("norm_w", [D], FP32, kind="ExternalInput").ap()
    out = nc.dram_tensor("out", [T, D], FP32, kind="ExternalOutput").ap()

    comb = nc.dram_tensor("comb_scratch", [TCOMB, D], FP32).ap()
    idxg_dram = nc.dram_tensor("idxg_scratch", [E, C], I16).ap()
    idxs_dram = nc.dram_tensor("idxs_scratch", [E, C], I16).ap()

    xt_r = xt.rearrange("(ko p) t -> p ko t", p=P)
    rw_r = router_w.rearrange("(ko p) e -> p ko e", p=P)
    pw_r = proj_w.rearrange("(ko p) c -> p ko c", p=P)
    comb_r = comb.rearrange("(n p) d -> p n d", p=P)

    with tile.TileContext(nc) as tc, ExitStack() as ctx:
        v = nc.vector
        s = nc.scalar

        singles = ctx.enter_context(tc.tile_pool(name="singles", bufs=1))
        idxp = ctx.enter_context(tc.tile_pool(name="idxp", bufs=1))

        # ---- resident small tensors ----
        rw_sb = singles.tile([P, KO, E], FP32)
        nc.sync.dma_start(out=rw_sb, in_=rw_r)
        rb_col = singles.tile([E, 1], FP32)
        nc.sync.dma_start(out=rb_col, in_=router_b[:, None])
        identity = singles.tile([P, P], FP32)
        make_identity(nc, identity)
        identity_bf = singles.tile([P, P], BF16)
        v.tensor_copy(out=identity_bf, in_=identity)
        eps_t = singles.tile([P, 1], FP32)
        v.memset(eps_t, EPS)
        iota1_i = singles.tile([P, C], I32)
        nc.gpsimd.iota(iota1_i, pattern=[[1, C]], base=1, channel_multiplier=0)
        iota1 = singles.tile([P, C], FP32)
        v.tensor_copy(out=iota1, in_=iota1_i)
        zero_big = singles.tile([P, D], FP32)
        v.memset(zero_big, 0.0)

        # zero the comb accumulator (incl. trash rows)
        nzc = TCOMB // P
        nc.scalar.dma_start(
            out=comb_r,
            in_=bass.AP(tensor=zero_big.tensor, offset=zero_big.offset,
                        ap=[list(zero_big.ap[0]), [0, nzc], [1, D]]),
        )

        # per-expert compact metadata (gates + replicated idx arrays)
        gate_sb = [idxp.tile([P, NCT], FP32, name=f"gate{e}") for e in range(E)]
        idxg_rep = [idxp.tile([P, NIC], I16, name=f"idxgr{e}") for e in range(E)]
        idxs_rep = [idxp.tile([P, NIC], I16, name=f"idxsr{e}") for e in range(E)]

        # =========== phase 1: router (true fp32) + top-2 gates ===========
        with (
            tc.tile_pool(name="xstream", bufs=3) as xsp,
            tc.tile_pool(name="psr", bufs=1, space="PSUM") as psr,
            tc.tile_pool(name="pst", bufs=2, space="PSUM") as pst,
            tc.tile_pool(name="rsm", bufs=2) as rsm,
            tc.tile_pool(name="rowp", bufs=1) as rowp,
            tc.tile_pool(name="tokp", bufs=1) as tokp,
            tc.tile_pool(name="gmat", bufs=1) as gmat,
            tc.tile_pool(name="psg", bufs=2, space="PSUM") as psg,
        ):
            lgT_ps = psr.tile([E, T], FP32)
            for ko in range(KO):
                xs = xsp.tile([P, T], FP32, tag="xs", name=f"xs{ko}")
                eng = nc.sync if ko % 2 == 0 else nc.scalar
                eng.dma_start(out=xs, in_=xt_r[:, ko, :])
                for nch in range(T // 512):
                    nc.tensor.matmul(
                        lgT_ps[:, nch * 512:(nch + 1) * 512],
                        lhsT=rw_sb[:, ko, :],
                        rhs=xs[:, nch * 512:(nch + 1) * 512],
                        start=(ko == 0),
                        stop=(ko == KO - 1),
                    )
            lg_row = rowp.tile([E, T], FP32)
            v.tensor_scalar(lg_row, lgT_ps, rb_col, None, op0=ALU.add)

            # token-major logits -> top-2 softmax gates per token tile
            fw_toks = []
            for tt in range(NTT):
                ps_l = pst.tile([P, E], FP32, tag="pstt")
                nc.tensor.transpose(ps_l, lg_row[:, tt * P:(tt + 1) * P],
                                    identity[:E, :E])
                logits = rsm.tile([P, E], FP32, tag="lg")
                v.tensor_copy(out=logits, in_=ps_l)
                m1 = rsm.tile([P, 1], FP32, tag="m1")
                v.tensor_reduce(m1, logits, axis=AX.X, op=ALU.max)
                mask1 = rsm.tile([P, E], FP32, tag="mk1")
                v.tensor_scalar(mask1, logits, m1, None, op0=ALU.is_ge)
                lg2 = rsm.tile([P, E], FP32, tag="lg2")
                v.scalar_tensor_tensor(
                    out=lg2, in0=mask1, scalar=NEG_BIG, in1=logits,
                    op0=ALU.mult, op1=ALU.add,
                )
                m2 = rsm.tile([P, 1], FP32, tag="m2")
                v.tensor_reduce(m2, lg2, axis=AX.X, op=ALU.max)
                mask2 = rsm.tile([P, E], FP32, tag="mk2")
                v.tensor_scalar(mask2, lg2, m2, None, op0=ALU.is_ge)
                d21 = rsm.tile([P, 1], FP32, tag="d21")
                v.tensor_tensor(out=d21, in0=m2, in1=m1, op=ALU.subtract)
                e2 = rsm.tile([P, 1], FP32, tag="e2")
                s.activation(e2, d21, ACTF.Exp)
                den = rsm.tile([P, 1], FP32, tag="den")
                v.tensor_scalar(den, e2, 1.0, None, op0=ALU.add)
                winv = rsm.tile([P, 1], FP32, tag="winv")
                v.reciprocal(winv, den)
                w2 = rsm.tile([P, 1], FP32, tag="w2")
                v.tensor_tensor(out=w2, in0=e2, in1=winv, op=ALU.mult)
                t2 = rsm.tile([P, E], FP32, tag="t2")
                v.tensor_scalar(t2, mask2, w2, None, op0=ALU.mult)
                fw = tokp.tile([P, E], FP32, name=f"fw{tt}")
                v.scalar_tensor_tensor(
                    out=fw, in0=mask1, scalar=winv, in1=t2,
                    op0=ALU.mult, op1=ALU.add,
                )
                fw_toks.append(fw)

            # masks + prefix-sum of routed counts in expert-major layout
            fwT = rowp.tile([E, T], FP32)
            for tt in range(NTT):
                ps_t = pst.tile([E, P], FP32, tag="pstt")
                nc.tensor.transpose(ps_t, fw_toks[tt], identity)
                v.tensor_copy(out=fwT[:, tt * P:(tt + 1) * P], in_=ps_t)
            mask_row = rowp.tile([E, T], FP32)
            v.tensor_scalar(mask_row, fwT, 0.0, None, op0=ALU.is_gt)
            zeros_row = rowp.tile([E, T], FP32)
            v.memset(zeros_row, 0.0)
            pos_row = rowp.tile([E, T], FP32)
            v.tensor_tensor_scan(
                out=pos_row, data0=mask_row, data1=zeros_row, initial=0.0,
                op0=ALU.add, op1=ALU.add,
            )

            # token-major views of pos and mask + matvec rhs per token tile
            pos_toks, mask_toks, rhs_toks = [], [], []
            for tt in range(NTT):
                ps_p = pst.tile([P, E], FP32, tag="pstt")
                nc.tensor.transpose(ps_p, pos_row[:, tt * P:(tt + 1) * P],
                                    identity[:E, :E])
                pos_t = tokp.tile([P, E], FP32, name=f"pos{tt}")
                v.tensor_copy(out=pos_t, in_=ps_p)
                pos_toks.append(pos_t)
                mk = tokp.tile([P, E], FP32, name=f"msk{tt}")
                v.tensor_scalar(mk, fw_toks[tt], 0.0, None, op0=ALU.is_gt)
                mask_toks.append(mk)
                rhs = tokp.tile([P, 2 + E], FP32, name=f"rhs{tt}")
                nc.gpsimd.iota(rhs[:, 0:1], pattern=[[0, 1]], base=tt * P,
                       channel_multiplier=1, allow_small_or_imprecise_dtypes=True)
                v.memset(rhs[:, 1:2], 1.0)
                v.tensor_copy(out=rhs[:, 2:2 + E], in_=fw_toks[tt])
                rhs_toks.append(rhs)

            # per-expert compact slots: G one-hot + matvec -> idx / valid / gate
            for e in range(E):
                g_tiles = []
                for tt in range(NTT):
                    g = gmat.tile([P, C], FP32, tag=f"g{tt}", name=f"g{e}_{tt}")
                    v.tensor_scalar(
                        g, iota1, pos_toks[tt][:, e:e + 1],
                        mask_toks[tt][:, e:e + 1],
                        op0=ALU.is_equal, op1=ALU.mult,
                    )
                    g_tiles.append(g)
                idxg_i = gmat.tile([P, NCT], I16, tag="ig", bufs=2, name=f"idxg{e}")
                idxs_i = gmat.tile([P, NCT], I16, tag="is", bufs=2, name=f"idxs{e}")
                for ct in range(NCT):
                    ps_mv = psg.tile([P, 2 + E], FP32, tag="psmv")
                    for tt in range(NTT):
                        nc.tensor.matmul(
                            ps_mv,
                            lhsT=g_tiles[tt][:, ct * P:(ct + 1) * P],
                            rhs=rhs_toks[tt],
                            start=(tt == 0),
                            stop=(tt == NTT - 1),
                        )
                    v.tensor_copy(out=idxg_i[:, ct:ct + 1], in_=ps_mv[:, 0:1])
                    # padding slots (valid==0) scatter to the trash row
                    tr = gmat.tile([P, 1], FP32, tag="tr", name=f"tr{e}_{ct}")
                    v.tensor_scalar(
                        tr, ps_mv[:, 1:2], float(-TRASH), float(TRASH),
                        op0=ALU.mult, op1=ALU.add,
                    )
                    v.tensor_tensor(out=tr, in0=tr, in1=ps_mv[:, 0:1], op=ALU.add)
                    v.tensor_copy(out=idxs_i[:, ct:ct + 1], in_=tr)
                    v.tensor_copy(
                        out=gate_sb[e][:, ct:ct + 1], in_=ps_mv[:, 2 + e:3 + e]
                    )
                # round-trip through DRAM into the swdge [16, C/16] layout,
                # replicated across the 8 partition groups
                weng = nc.sync if e % 2 == 0 else nc.scalar
                weng.dma_start(
                    out=idxg_dram[e].rearrange("(b p) -> p b", p=P), in_=idxg_i
                )
                weng.dma_start(
                    out=idxs_dram[e].rearrange("(b p) -> p b", p=P), in_=idxs_i
                )
                for g in range(8):
                    geng = nc.sync if g % 2 == 0 else nc.scalar
                    geng.dma_start(
                        out=idxg_rep[e][g * 16:(g + 1) * 16, :],
                        in_=bass.AP(tensor=idxg_dram.tensor, offset=e * C,
                                    ap=[[1, 16], [16, NIC]]),
                    )
                    geng.dma_start(
                        out=idxs_rep[e][g * 16:(g + 1) * 16, :],
                        in_=bass.AP(tensor=idxs_dram.tensor, offset=e * C,
                                    ap=[[1, 16], [16, NIC]]),
                    )

        # =========== phase 2: sparse expert compute ===========
        expert_ctx = ExitStack()
        xg_pool = expert_ctx.enter_context(tc.tile_pool(name="xg", bufs=2))
        w_pool = expert_ctx.enter_context(tc.tile_pool(name="wp", bufs=2))
        h_pool = expert_ctx.enter_context(tc.tile_pool(name="hp", bufs=2))
        eb_pool = expert_ctx.enter_context(tc.tile_pool(name="ebp", bufs=2))
        sil_pool = expert_ctx.enter_context(tc.tile_pool(name="silp", bufs=4))
        pse = tc.alloc_tile_pool(name="pse", bufs=6, space="PSUM")

        for e in range(E):
            xgT = xg_pool.tile([P, KO, C], BF16, tag="xgT", name=f"xgT{e}")
            nc.gpsimd.dma_gather(
                xgT[:],
                xrb,
                idxg_rep[e][:],
                C,
                C,
                D,
                elem_step=D,
                transpose=True,
                queue_num=0,
            )
            eb_rep = eb_pool.tile([P, D], FP32, tag="eb", name=f"eb{e}")
            nc.scalar.dma_start(out=eb_rep, in_=_bcast_ap(expert_b[e]))
            we_r = expert_w[e].rearrange("(ko p) c -> p ko c", p=P)
            h_silu = h_pool.tile([P, NCT, D], FP32, tag="hs", name=f"hs{e}")
            for half in range(NH):
                wh = w_pool.tile([P, KO, HW], BF16, tag="wh", name=f"wh{e}_{half}")
                weng = nc.sync if (e + half) % 2 == 0 else nc.scalar
                weng.dma_start(out=wh, in_=we_r[:, :, half * HW:(half + 1) * HW])
                for ct in range(NCT):
                    pss = [
                        pse.tile([P, 512], FP32, tag="ps",
                                 name=f"ps{e}_{half}_{ct}_{i}")
                        for i in range(HW // 512)
                    ]
                    for ko in range(KO):
                        for i, ps in enumerate(pss):
                            nc.tensor.matmul(
                                ps,
                                lhsT=xgT[:, ko, ct * P:(ct + 1) * P],
                                rhs=wh[:, ko, i * 512:(i + 1) * 512],
                                start=(ko == 0),
                                stop=(ko == KO - 1),
                            )
                    for i, ps in enumerate(pss):
                        fo = half * HW + i * 512
                        t1 = sil_pool.tile([P, 512], FP32, tag="t1",
                                           name=f"t1{e}_{half}_{ct}_{i}")
                        v.tensor_tensor(out=t1, in0=ps, in1=eb_rep[:, fo:fo + 512],
                                        op=ALU.add)
                        sg = sil_pool.tile([P, 512], FP32, tag="sg",
                                           name=f"sg{e}_{half}_{ct}_{i}")
                        s.activation(sg, t1, ACTF.Sigmoid)
                        v.scalar_tensor_tensor(
                            out=h_silu[:, ct, fo:fo + 512],
                            in0=t1, scalar=gate_sb[e][:, ct:ct + 1], in1=sg,
                            op0=ALU.mult, op1=ALU.mult,
                        )
            nc.gpsimd.dma_scatter_add(
                comb,
                h_silu[:],
                idxs_rep[e][:],
                C,
                C,
                D,
                queue_num=0,
            )

        pse.release()
        expert_ctx.close()

        # =========== phase 3: projection + residual + RMSNorm ===========
        with (
            tc.tile_pool(name="p3s", bufs=1) as p3s,
            tc.tile_pool(name="cbp", bufs=2) as cbp,
            tc.tile_pool(name="ctp", bufs=1) as ctp,
            tc.tile_pool(name="yp", bufs=2) as yp,
            tc.tile_pool(name="xresp", bufs=2) as xresp,
            tc.tile_pool(name="nsm", bufs=2) as nsm,
        ):
            pw_sb = p3s.tile([P, KO, D], BF16)
            for ko in range(KO):
                eng = nc.sync if ko % 2 == 0 else nc.scalar
                eng.dma_start(out=pw_sb[:, ko, :], in_=pw_r[:, ko, :])
            nw_rep = p3s.tile([P, D], FP32)
            nc.sync.dma_start(out=nw_rep, in_=_bcast_ap(norm_w))
            prb = p3s.tile([P, D], FP32)
            nc.sync.dma_start(out=prb, in_=_bcast_ap(proj_b))

            combT = ctp.tile([P, KO, T], BF16)
            with tc.tile_pool(name="pstc", bufs=4, space="PSUM") as pstc:
                for tt in range(NTT):
                    cb = cbp.tile([P, D], FP32, tag="cb", name=f"cb{tt}")
                    eng = nc.sync if tt % 2 == 0 else nc.scalar
                    eng.dma_start(out=cb, in_=comb[tt * P:(tt + 1) * P, :])
                    cbb = cbp.tile([P, D], BF16, tag="cbb", name=f"cbb{tt}")
                    v.tensor_copy(out=cbb, in_=cb)
                    for ko in range(KO):
                        ps_t = pstc.tile([P, P], BF16, tag="pst")
                        nc.tensor.transpose(
                            ps_t, cbb[:, ko * P:(ko + 1) * P], identity_bf
                        )
                        v.tensor_copy(out=combT[:, ko, tt * P:(tt + 1) * P],
                                      in_=ps_t)

            psp = tc.alloc_tile_pool(name="psp", bufs=8, space="PSUM")
            HD = D // 2

            def emit_norm(tt, y_t):
                sq = nsm.tile([P, HD], FP32, tag="sq", bufs=1, name=f"sq{tt}")
                ssa = nsm.tile([P, 1], FP32, tag="ssa", name=f"ssa{tt}")
                ssb = nsm.tile([P, 1], FP32, tag="ssb", name=f"ssb{tt}")
                s.activation(sq, y_t[:, :HD], ACTF.Square, accum_out=ssa)
                s.activation(sq, y_t[:, HD:], ACTF.Square, accum_out=ssb)
                ssum = nsm.tile([P, 1], FP32, tag="ssum", name=f"ssum{tt}")
                v.tensor_tensor(out=ssum, in0=ssa, in1=ssb, op=ALU.add)
                rms = nsm.tile([P, 1], FP32, tag="rms", name=f"rms{tt}")
                s.activation(rms, ssum, ACTF.Sqrt, bias=eps_t, scale=1.0 / D)
                rinv = nsm.tile([P, 1], FP32, tag="rinv", name=f"rinv{tt}")
                v.reciprocal(rinv, rms)
                s.mul(y_t, y_t, rinv)
                v.tensor_tensor(out=y_t, in0=y_t, in1=nw_rep, op=ALU.mult)
                oeng = nc.sync if tt % 2 == 0 else nc.scalar
                oeng.dma_start(out=out[tt * P:(tt + 1) * P, :], in_=y_t)

            for tt in range(NTT):
                pso = [
                    psp.tile([P, 512], FP32, tag="pso", name=f"pso{tt}_{i}")
                    for i in range(D // 512)
                ]
                for ko in range(KO):
                    for i, ps in enumerate(pso):
                        nc.tensor.matmul(
                            ps,
                            lhsT=combT[:, ko, tt * P:(tt + 1) * P],
                            rhs=pw_sb[:, ko, i * 512:(i + 1) * 512],
                            start=(ko == 0),
                            stop=(ko == KO - 1),
                        )
                xres = xresp.tile([P, D], FP32, tag="xres", name=f"xres{tt}")
                nc.scalar.dma_start(out=xres, in_=xr[tt * P:(tt + 1) * P, :])
                y_t = yp.tile([P, D], FP32, tag="y", name=f"y{tt}")
                for i, ps in enumerate(pso):
                    y_sl = y_t[:, i * 512:(i + 1) * 512]
                    v.tensor_tensor(out=y_sl, in0=ps, in1=prb[:, i * 512:(i + 1) * 512],
                                    op=ALU.add)
                    v.tensor_tensor(out=y_sl, in0=y_sl,
                                    in1=xres[:, i * 512:(i + 1) * 512], op=ALU.add)
                emit_norm(tt, y_t)
            psp.release()

    nc.compile()
    return nc


# ---- full-problem entry point ----
_B, _S, _D, _E = 4, 2048, 2048, 8
_NCORES = 8
_T = _B * _S // _NCORES
_C = 384

_nc_cache = None


def _get_nc():
    global _nc_cache
    if _nc_cache is None:
        _nc_cache = build_moe_v2(_D, _E, _T, C=_C)
    return _nc_cache


def _make_in_maps(xf, router_w, router_b, expert_w, expert_b, proj_w, proj_b,
                  norm_w):
    import ml_dtypes
    ew_bf = np.ascontiguousarray(expert_w.astype(ml_dtypes.bfloat16))
    pw_bf = np.ascontiguousarray(proj_w.astype(ml_dtypes.bfloat16))
    in_maps = []
    for c in range(_NCORES):
        xs = xf[c * _T:(c + 1) * _T]
        m = {
            "xt": np.ascontiguousarray(xs.T),
            "xrb": np.ascontiguousarray(xs.astype(ml_dtypes.bfloat16)),
            "xr": np.ascontiguousarray(xs),
            "router_w": router_w,
            "router_b": router_b,
            "expert_w": ew_bf,
            "expert_b": expert_b,
            "proj_w": pw_bf,
            "proj_b": proj_b,
            "norm_w": norm_w,
        }
        in_maps.append(m)
    return in_maps


def kernel(x, router_w, router_b, expert_w, expert_b, proj_w, proj_b, norm_w):
    from concourse import bass_utils

    x = np.asarray(x, np.float32)
    router_w = np.asarray(router_w, np.float32)
    router_b = np.asarray(router_b, np.float32)
    expert_w = np.asarray(expert_w, np.float32)
    expert_b = np.asarray(expert_b, np.float32)
    proj_w = np.asarray(proj_w, np.float32)
    proj_b = np.asarray(proj_b, np.float32)
    norm_w = np.asarray(norm_w, np.float32)

    nc = _get_nc()
    xf = x.reshape(-1, _D)
    in_maps = _make_in_maps(xf, router_w, router_b, expert_w, expert_b,
                            proj_w, proj_b, norm_w)
    res = bass_utils.run_bass_kernel_spmd(nc, in_maps, core_ids=list(range(_NCORES)))
    outs = [res.results[c]["out"] for c in range(_NCORES)]
    return np.concatenate(outs, axis=0).reshape(_B, _S, _D).astype(np.float32)


# revision 3
# speedup vs baseline: 1.1564x; 1.0258x over previous
"""Trainium2 Bass kernel for EnhancedGatedFusion (MoE top-2 of 8), v2: sparse dispatch.

Strategy: data-parallel over tokens across 8 cores (T=1024 tokens each).
Per core:
  1. Router in true fp32 (top-2 selection is precision critical): stream
     x^T ko-slices, matmul vs router_w -> logits^T [E, T] in PSUM, add bias,
     transpose to token-major, dense top-2 softmax via max/mask trick.
  2. Build per-expert compact token lists ON DEVICE with no dynamic
     addressing: prefix-sum the routed-mask along tokens (tensor_tensor_scan),
     form one-hot G[t, slot] tiles via iota/is_equal, then tiny matmuls
     G^T @ [t, 1, gates] give, per compact slot: token idx, validity, gate.
     Indices round-trip through DRAM into the [16, n/16] int16 layout the
     SWDGE gather/scatter ops require (replicated across 8 partition groups).
  3. dma_gather(transpose=True) fetches the C=384-padded token set for each
     expert from token-major bf16 x in DRAM, landing feature-major
     [128, KO, C] -- the exact lhsT layout for the expert matmul.
  4. Expert matmul (bf16): x-compact stationary, expert_w streamed in halves;
     silu(h + b) * gate on scalar/vector engines (padding slots have gate 0).
  5. dma_scatter_add accumulates contributions into a DRAM comb buffer
     (fp32); padding slots carry exact zeros and are directed at trash rows
     past the real tokens so their adds cannot race real rows.
  6. comb is read back, converted bf16, PE-transposed, and projected
     (bf16) + bias + residual + RMSNorm as in the dense baseline.
"""

import sys

for _p in ("/opt/trn_rl_repo",):
    if _p not in sys.path:
        sys.path.insert(0, _p)

from contextlib import ExitStack

import numpy as np

import concourse.bass as bass
import concourse.mybir as mybir
import concourse.tile as tile
from concourse import bacc
from concourse.masks import make_identity

FP32 = mybir.dt.float32
FP32R = mybir.dt.float32r
BF16 = mybir.dt.bfloat16
I16 = mybir.dt.int16
I32 = mybir.dt.int32
AX = mybir.AxisListType
ALU = mybir.AluOpType
ACTF = mybir.ActivationFunctionType

EPS = 1e-6
NEG_BIG = -1e30


def _bcast_ap(ap, nparts=128):
    """Partition-broadcast view of a DRAM AP (step-0 partition dim)."""
    return bass.AP(tensor=ap.tensor, offset=ap.offset, ap=[[0, nparts], *ap.ap])


def build_moe_v2(D, E, T, C=384, trn_type="TRN2"):
    """Per-core sparse MoE program. D model dim, E experts, T tokens/core,
    C compact capacity per expert (multiple of 128 slots, >= max expert load)."""
    P = 128
    KO = D // P            # contraction k-tiles
    NTT = T // P           # token tiles
    NCT = C // P           # compact c-tiles per expert
    NIC = C // 16          # idx columns in the [16, C/16] swdge layout
    TRASH = T              # scatter row for padding slots
    TCOMB = ((T + C) + P - 1) // P * P  # comb rows incl. trash, multiple of 128
    NH = 2                 # expert_w streamed in halves
    HW = D // NH           # half width

    nc = bacc.Bacc(trn_type, target_bir_lowering=False, debug=False)

    xt = nc.dram_tensor("xt", [D, T], FP32, kind="ExternalInput").ap()
    xrb = nc.dram_tensor("xrb", [T, D], BF16, kind="ExternalInput").ap()
    xr = nc.dram_tensor("xr", [T, D], FP32, kind="ExternalInput").ap()
    router_w = nc.dram_tensor("router_w", [D, E], FP32, kind="ExternalInput").ap()
    router_b = nc.dram_tensor("router_b", [E], FP32, kind="ExternalInput").ap()
    expert_w = nc.dram_tensor("expert_w", [E, D, D], BF16, kind="ExternalInput").ap()
    expert_b = nc.dram_tensor("expert_b", [E, D], FP32, kind="ExternalInput").ap()
    proj_w = nc.dram_tensor("proj_w", [D, D], BF16, kind="ExternalInput").ap()
    proj_b = nc.dram_tensor("proj_b", [D], FP32, kind="ExternalInput").ap()
    norm_w = nc.dram_tensor("norm_w", [D], FP32, kind="ExternalInput").ap()
    out = nc.dram_tensor("out", [T, D], FP32, kind="ExternalOutput").ap()

    comb = nc.dram_tensor("comb_scratch", [TCOMB, D], FP32).ap()
    idxg_dram = nc.dram_tensor("idxg_scratch", [E, C], I16).ap()
    idxs_dram = nc.dram_tensor("idxs_scratch", [E, C], I16).ap()

    xt_r = xt.rearrange("(ko p) t -> p ko t", p=P)
    rw_r = router_w.rearrange("(ko p) e -> p ko e", p=P)
    pw_r = proj_w.rearrange("(ko p) c -> p ko c", p=P)
    comb_r = comb.rearrange("(n p) d -> p n d", p=P)

    with tile.TileContext(nc) as tc, ExitStack() as ctx:
        v = nc.vector
        s = nc.scalar

        singles = ctx.enter_context(tc.tile_pool(name="singles", bufs=1))
        idxp = ctx.enter_context(tc.tile_pool(name="idxp", bufs=1))

        # ---- resident small tensors ----
        rw_sb = singles.tile([P, KO, E], FP32)
        nc.sync.dma_start(out=rw_sb, in_=rw_r)
        rb_col = singles.tile([E, 1], FP32)
        nc.sync.dma_start(out=rb_col, in_=router_b[:, None])
        identity = singles.tile([P, P], FP32)
        make_identity(nc, identity)
        identity_bf = singles.tile([P, P], BF16)
        v.tensor_copy(out=identity_bf, in_=identity)
        eps_t = singles.tile([P, 1], FP32)
        v.memset(eps_t, EPS)
        iota1_i = singles.tile([P, C], I32)
        nc.gpsimd.iota(iota1_i, pattern=[[1, C]], base=1, channel_multiplier=0)
        iota1 = singles.tile([P, C], FP32)
        v.tensor_copy(out=iota1, in_=iota1_i)
        zero_big = singles.tile([P, D], FP32)
        v.memset(zero_big, 0.0)

        # zero the comb accumulator (incl. trash rows)
        nzc = TCOMB // P
        nc.scalar.dma_start(
            out=comb_r,
            in_=bass.AP(tensor=zero_big.tensor, offset=zero_big.offset,
                        ap=[list(zero_big.ap[0]), [0, nzc], [1, D]]),
        )

        # per-expert compact metadata (gates + replicated idx arrays)
        gate_sb = [idxp.tile([P, NCT], FP32, name=f"gate{e}") for e in range(E)]
        idxg_rep = idxp.tile([P, E, NIC], I16, name="idxgr")
        idxs_rep = idxp.tile([P, E, NIC], I16, name="idxsr")

        # =========== phase 1: router (true fp32) + top-2 gates ===========
        with (
            tc.tile_pool(name="xstream", bufs=3) as xsp,
            tc.tile_pool(name="psr", bufs=1, space="PSUM") as psr,
            tc.tile_pool(name="pst", bufs=2, space="PSUM") as pst,
            tc.tile_pool(name="rsm", bufs=2) as rsm,
            tc.tile_pool(name="rowp", bufs=1) as rowp,
            tc.tile_pool(name="tokp", bufs=1) as tokp,
            tc.tile_pool(name="gmat", bufs=1) as gmat,
            tc.tile_pool(name="psg", bufs=2, space="PSUM") as psg,
        ):
            lgT_ps = psr.tile([E, T], FP32)
            for ko in range(KO):
                xs = xsp.tile([P, T], FP32, tag="xs", name=f"xs{ko}")
                eng = nc.sync if ko % 2 == 0 else nc.scalar
                eng.dma_start(out=xs, in_=xt_r[:, ko, :])
                for nch in range(T // 512):
                    nc.tensor.matmul(
                        lgT_ps[:, nch * 512:(nch + 1) * 512],
                        lhsT=rw_sb[:, ko, :],
                        rhs=xs[:, nch * 512:(nch + 1) * 512],
                        start=(ko == 0),
                        stop=(ko == KO - 1),
                    )
            lg_row = rowp.tile([E, T], FP32)
            v.tensor_scalar(lg_row, lgT_ps, rb_col, None, op0=ALU.add)

            def _fbc(ap, n):
                # broadcast view: append a step-0 innermost dim of size n
                return bass.AP(tensor=ap.tensor, offset=ap.offset,
                               ap=[*ap.ap, [0, n]])

            # token-major logits, all tiles batched: [P, NTT, E]
            lg_all = tokp.tile([P, NTT, E], FP32, name="lg_all")
            for tt in range(NTT):
                ps_l = pst.tile([P, E], FP32, tag="pstt")
                nc.tensor.transpose(ps_l, lg_row[:, tt * P:(tt + 1) * P],
                                    identity[:E, :E])
                v.tensor_copy(out=lg_all[:, tt, :], in_=ps_l)

            # batched top-2 softmax over the innermost (expert) axis
            m1 = tokp.tile([P, NTT], FP32, name="m1")
            v.tensor_reduce(m1, lg_all, axis=AX.X, op=ALU.max)
            mask1 = tokp.tile([P, NTT, E], FP32, name="mask1")
            v.tensor_tensor(out=mask1, in0=lg_all, in1=_fbc(m1, E), op=ALU.is_ge)
            lg2 = tokp.tile([P, NTT, E], FP32, name="lg2")
            v.scalar_tensor_tensor(out=lg2, in0=mask1, scalar=NEG_BIG,
                                   in1=lg_all, op0=ALU.mult, op1=ALU.add)
            m2 = tokp.tile([P, NTT], FP32, name="m2")
            v.tensor_reduce(m2, lg2, axis=AX.X, op=ALU.max)
            mask2 = tokp.tile([P, NTT, E], FP32, name="mask2")
            v.tensor_tensor(out=mask2, in0=lg2, in1=_fbc(m2, E), op=ALU.is_ge)
            d21 = tokp.tile([P, NTT], FP32, name="d21")
            v.tensor_tensor(out=d21, in0=m2, in1=m1, op=ALU.subtract)
            e2 = tokp.tile([P, NTT], FP32, name="e2")
            s.activation(e2, d21, ACTF.Exp)
            den = tokp.tile([P, NTT], FP32, name="den")
            v.tensor_scalar(den, e2, 1.0, None, op0=ALU.add)
            winv = tokp.tile([P, NTT], FP32, name="winv")
            v.reciprocal(winv, den)
            w2 = tokp.tile([P, NTT], FP32, name="w2")
            v.tensor_tensor(out=w2, in0=e2, in1=winv, op=ALU.mult)
            t2 = tokp.tile([P, NTT, E], FP32, name="t2")
            v.tensor_tensor(out=t2, in0=mask2, in1=_fbc(w2, E), op=ALU.mult)
            fw_all = tokp.tile([P, NTT, E], FP32, name="fw_all")
            v.tensor_tensor(out=fw_all, in0=mask1, in1=_fbc(winv, E), op=ALU.mult)
            v.tensor_tensor(out=fw_all, in0=fw_all, in1=t2, op=ALU.add)
            mask_all = tokp.tile([P, NTT, E], FP32, name="mask_all")
            v.tensor_scalar(mask_all, fw_all, 0.0, None, op0=ALU.is_gt)

            # expert-major gate rows -> routed-mask prefix sums
            fwT = rowp.tile([E, T], FP32)
            for tt in range(NTT):
                ps_t = pst.tile([E, P], FP32, tag="pstt")
                nc.tensor.transpose(ps_t, fw_all[:, tt, :], identity)
                v.tensor_copy(out=fwT[:, tt * P:(tt + 1) * P], in_=ps_t)
            mask_row = rowp.tile([E, T], FP32)
            v.tensor_scalar(mask_row, fwT, 0.0, None, op0=ALU.is_gt)
            zeros_row = rowp.tile([E, T], FP32)
            v.memset(zeros_row, 0.0)
            pos_row = rowp.tile([E, T], FP32)
            v.tensor_tensor_scan(
                out=pos_row, data0=mask_row, data1=zeros_row, initial=0.0,
                op0=ALU.add, op1=ALU.add,
            )
            pos_all = tokp.tile([P, NTT, E], FP32, name="pos_all")
            for tt in range(NTT):
                ps_p = pst.tile([P, E], FP32, tag="pstt")
                nc.tensor.transpose(ps_p, pos_row[:, tt * P:(tt + 1) * P],
                                    identity[:E, :E])
                v.tensor_copy(out=pos_all[:, tt, :], in_=ps_p)

            # matvec rhs: [tvec, ones, gates] per token tile, built batched
            rhs_all = tokp.tile([P, NTT, 2 + E], FP32, name="rhs_all")
            nc.gpsimd.iota(rhs_all[:, :, 0:1], pattern=[[P, NTT], [0, 1]],
                           base=0, channel_multiplier=1,
                           allow_small_or_imprecise_dtypes=True)
            v.memset(rhs_all[:, :, 1:2], 1.0)
            v.tensor_copy(out=rhs_all[:, :, 2:2 + E], in_=fw_all)

            # one-hot G per expert (batched over token tiles) + matvecs into
            # a single PSUM strip; slot c of expert e gets (idx, valid, gates)
            iota_big = gmat.tile([P, NTT, C], FP32, name="iota_big")
            v.tensor_copy(
                out=iota_big,
                in_=bass.AP(tensor=iota1.tensor, offset=iota1.offset,
                            ap=[list(iota1.ap[0]), [0, NTT], [1, C]]),
            )
            mv_ps = psg.tile([P, E * NCT * (2 + E)], FP32, name="mv_ps")
            for e in range(E):
                g_all = gmat.tile([P, NTT, C], FP32, tag="g_all",
                                  bufs=2, name=f"g{e}")
                pos_e = bass.AP(tensor=pos_all.tensor,
                                offset=pos_all.offset + e,
                                ap=[list(pos_all.ap[0]), [E, NTT], [0, C]])
                mask_e = bass.AP(tensor=mask_all.tensor,
                                 offset=mask_all.offset + e,
                                 ap=[list(mask_all.ap[0]), [E, NTT], [0, C]])
                v.tensor_tensor(out=g_all, in0=iota_big, in1=pos_e,
                                op=ALU.is_equal)
                v.tensor_tensor(out=g_all, in0=g_all, in1=mask_e, op=ALU.mult)
                for ct in range(NCT):
                    col = (e * NCT + ct) * (2 + E)
                    for tt in range(NTT):
                        nc.tensor.matmul(
                            mv_ps[:, col:col + 2 + E],
                            lhsT=g_all[:, tt, ct * P:(ct + 1) * P],
                            rhs=rhs_all[:, tt, :],
                            start=(tt == 0),
                            stop=(tt == NTT - 1),
                        )
            mv_all = gmat.tile([P, E, NCT, 2 + E], FP32, name="mv_all")
            v.tensor_copy(out=mv_all, in_=mv_ps)

            # batched idx/gate extraction
            idxg_i = gmat.tile([P, E * NCT], I16, name="idxg_i")
            v.tensor_copy(out=idxg_i, in_=mv_all[:, :, :, 0])
            trf = gmat.tile([P, E, NCT], FP32, name="trf")
            v.tensor_scalar(trf, mv_all[:, :, :, 1], float(-TRASH),
                            float(TRASH), op0=ALU.mult, op1=ALU.add)
            v.tensor_tensor(out=trf, in0=trf, in1=mv_all[:, :, :, 0],
                            op=ALU.add)
            idxs_i = gmat.tile([P, E * NCT], I16, name="idxs_i")
            v.tensor_copy(out=idxs_i, in_=trf)
            for e in range(E):
                v.tensor_copy(out=gate_sb[e], in_=mv_all[:, e, :, 2 + e])

            # DRAM round trip into the swdge [16, C/16] idx layout,
            # replicated across the 8 partition groups (one DMA per group)
            nc.sync.dma_start(
                out=bass.AP(tensor=idxg_dram.tensor, offset=0,
                            ap=[[1, P], [C, E], [P, NCT]]),
                in_=idxg_i.rearrange("p (e b) -> p e b", e=E),
            )
            nc.scalar.dma_start(
                out=bass.AP(tensor=idxs_dram.tensor, offset=0,
                            ap=[[1, P], [C, E], [P, NCT]]),
                in_=idxs_i.rearrange("p (e b) -> p e b", e=E),
            )
            for g in range(8):
                geng = nc.sync if g % 2 == 0 else nc.scalar
                geng.dma_start(
                    out=idxg_rep[g * 16:(g + 1) * 16, :, :],
                    in_=bass.AP(tensor=idxg_dram.tensor, offset=0,
                                ap=[[1, 16], [C, E], [16, NIC]]),
                )
                geng.dma_start(
                    out=idxs_rep[g * 16:(g + 1) * 16, :, :],
                    in_=bass.AP(tensor=idxs_dram.tensor, offset=0,
                                ap=[[1, 16], [C, E], [16, NIC]]),
                )

        # =========== phase 2: sparse expert compute ===========
        expert_ctx = ExitStack()
        xg_pool = expert_ctx.enter_context(tc.tile_pool(name="xg", bufs=2))
        w_pool = expert_ctx.enter_context(tc.tile_pool(name="wp", bufs=2))
        h_pool = expert_ctx.enter_context(tc.tile_pool(name="hp", bufs=2))
        eb_pool = expert_ctx.enter_context(tc.tile_pool(name="ebp", bufs=2))
        sil_pool = expert_ctx.enter_context(tc.tile_pool(name="silp", bufs=4))
        pse = tc.alloc_tile_pool(name="pse", bufs=6, space="PSUM")

        def emit_gather(e):
            xgT = xg_pool.tile([P, KO, C], BF16, tag="xgT", name=f"xgT{e}")
            nc.gpsimd.dma_gather(
                xgT[:],
                xrb,
                idxg_rep[:, e, :],
                C,
                C,
                D,
                elem_step=D,
                transpose=True,
                queue_num=0,
            )
            return xgT

        xgT_next = emit_gather(0)
        for e in range(E):
            xgT = xgT_next
            eb_rep = eb_pool.tile([P, D], FP32, tag="eb", name=f"eb{e}")
            nc.scalar.dma_start(out=eb_rep, in_=_bcast_ap(expert_b[e]))
            we_r = expert_w[e].rearrange("(ko p) c -> p ko c", p=P)
            h_silu = h_pool.tile([P, NCT, D], FP32, tag="hs", name=f"hs{e}")
            for half in range(NH):
                wh = w_pool.tile([P, KO, HW], BF16, tag="wh", name=f"wh{e}_{half}")
                weng = nc.sync if (e + half) % 2 == 0 else nc.scalar
                weng.dma_start(out=wh, in_=we_r[:, :, half * HW:(half + 1) * HW])
                for ct in range(NCT):
                    pss = [
                        pse.tile([P, 512], FP32, tag="ps",
                                 name=f"ps{e}_{half}_{ct}_{i}")
                        for i in range(HW // 512)
                    ]
                    for ko in range(KO):
                        for i, ps in enumerate(pss):
                            nc.tensor.matmul(
                                ps,
                                lhsT=xgT[:, ko, ct * P:(ct + 1) * P],
                                rhs=wh[:, ko, i * 512:(i + 1) * 512],
                                start=(ko == 0),
                                stop=(ko == KO - 1),
                            )
                    for i, ps in enumerate(pss):
                        fo = half * HW + i * 512
                        t1 = sil_pool.tile([P, 512], FP32, tag="t1",
                                           name=f"t1{e}_{half}_{ct}_{i}")
                        v.tensor_tensor(out=t1, in0=ps, in1=eb_rep[:, fo:fo + 512],
                                        op=ALU.add)
                        sg = sil_pool.tile([P, 512], FP32, tag="sg",
                                           name=f"sg{e}_{half}_{ct}_{i}")
                        s.activation(sg, t1, ACTF.Sigmoid)
                        v.scalar_tensor_tensor(
                            out=h_silu[:, ct, fo:fo + 512],
                            in0=t1, scalar=gate_sb[e][:, ct:ct + 1], in1=sg,
                            op0=ALU.mult, op1=ALU.mult,
                        )
            if e + 1 < E:
                xgT_next = emit_gather(e + 1)
            nc.gpsimd.dma_scatter_add(
                comb,
                h_silu[:],
                idxs_rep[:, e, :],
                C,
                C,
                D,
                queue_num=0,
            )

        pse.release()
        expert_ctx.close()

        # =========== phase 3: projection + residual + RMSNorm ===========
        with (
            tc.tile_pool(name="p3s", bufs=1) as p3s,
            tc.tile_pool(name="cbp", bufs=2) as cbp,
            tc.tile_pool(name="ctp", bufs=1) as ctp,
            tc.tile_pool(name="yp", bufs=2) as yp,
            tc.tile_pool(name="xresp", bufs=2) as xresp,
            tc.tile_pool(name="nsm", bufs=2) as nsm,
        ):
            pw_sb = p3s.tile([P, KO, D], BF16)
            for ko in range(KO):
                nc.sync.dma_start(out=pw_sb[:, ko, :], in_=pw_r[:, ko, :])
            nw_rep = p3s.tile([P, D], FP32)
            nc.sync.dma_start(out=nw_rep, in_=_bcast_ap(norm_w))
            prb = p3s.tile([P, D], FP32)
            nc.sync.dma_start(out=prb, in_=_bcast_ap(proj_b))

            combT = ctp.tile([P, KO, T], BF16)
            with tc.tile_pool(name="pstc", bufs=4, space="PSUM") as pstc:
                for tt in range(NTT):
                    cb = cbp.tile([P, D], FP32, tag="cb", name=f"cb{tt}")
                    nc.scalar.dma_start(out=cb, in_=comb[tt * P:(tt + 1) * P, :])
                    cbb = cbp.tile([P, D], BF16, tag="cbb", name=f"cbb{tt}")
                    v.tensor_copy(out=cbb, in_=cb)
                    for k4 in range(KO // 4):
                        ps_t = pstc.tile([P, 4, P], BF16, tag="pst")
                        for j in range(4):
                            ko = k4 * 4 + j
                            nc.tensor.transpose(
                                ps_t[:, j, :], cbb[:, ko * P:(ko + 1) * P],
                                identity_bf
                            )
                        v.tensor_copy(
                            out=combT[:, k4 * 4:(k4 + 1) * 4,
                                      tt * P:(tt + 1) * P],
                            in_=ps_t,
                        )

            psp = tc.alloc_tile_pool(name="psp", bufs=8, space="PSUM")
            HD = D // 2

            def emit_norm(tt, y_t):
                sq = nsm.tile([P, HD], FP32, tag="sq", bufs=1, name=f"sq{tt}")
                ssa = nsm.tile([P, 1], FP32, tag="ssa", name=f"ssa{tt}")
                ssb = nsm.tile([P, 1], FP32, tag="ssb", name=f"ssb{tt}")
                s.activation(sq, y_t[:, :HD], ACTF.Square, accum_out=ssa)
                s.activation(sq, y_t[:, HD:], ACTF.Square, accum_out=ssb)
                ssum = nsm.tile([P, 1], FP32, tag="ssum", name=f"ssum{tt}")
                v.tensor_tensor(out=ssum, in0=ssa, in1=ssb, op=ALU.add)
                rms = nsm.tile([P, 1], FP32, tag="rms", name=f"rms{tt}")
                s.activation(rms, ssum, ACTF.Sqrt, bias=eps_t, scale=1.0 / D)
                rinv = nsm.tile([P, 1], FP32, tag="rinv", name=f"rinv{tt}")
                v.reciprocal(rinv, rms)
                s.mul(y_t, y_t, rinv)
                v.tensor_tensor(out=y_t, in0=y_t, in1=nw_rep, op=ALU.mult)
                oeng = nc.sync if tt % 2 == 0 else nc.scalar
                oeng.dma_start(out=out[tt * P:(tt + 1) * P, :], in_=y_t)

            for tt in range(NTT):
                pso = [
                    psp.tile([P, 512], FP32, tag="pso", name=f"pso{tt}_{i}")
                    for i in range(D // 512)
                ]
                for ko in range(KO):
                    for i, ps in enumerate(pso):
                        nc.tensor.matmul(
                            ps,
                            lhsT=combT[:, ko, tt * P:(tt + 1) * P],
                            rhs=pw_sb[:, ko, i * 512:(i + 1) * 512],
                            start=(ko == 0),
                            stop=(ko == KO - 1),
                        )
                xres = xresp.tile([P, D], FP32, tag="xres", name=f"xres{tt}")
                nc.scalar.dma_start(out=xres, in_=xr[tt * P:(tt + 1) * P, :])
                y_t = yp.tile([P, D], FP32, tag="y", name=f"y{tt}")
                for i, ps in enumerate(pso):
                    y_sl = y_t[:, i * 512:(i + 1) * 512]
                    v.tensor_tensor(out=y_sl, in0=ps, in1=prb[:, i * 512:(i + 1) * 512],
                                    op=ALU.add)
                    v.tensor_tensor(out=y_sl, in0=y_sl,
                                    in1=xres[:, i * 512:(i + 1) * 512], op=ALU.add)
                emit_norm(tt, y_t)
            psp.release()

    nc.compile()
    return nc


# ---- full-problem entry point ----
_B, _S, _D, _E = 4, 2048, 2048, 8
_NCORES = 8
_T = _B * _S // _NCORES
_C = 384

_nc_cache = None


def _get_nc():
    global _nc_cache
    if _nc_cache is None:
        _nc_cache = build_moe_v2(_D, _E, _T, C=_C)
    return _nc_cache


def _make_in_maps(xf, router_w, router_b, expert_w, expert_b, proj_w, proj_b,
                  norm_w):
    import ml_dtypes
    ew_bf = np.ascontiguousarray(expert_w.astype(ml_dtypes.bfloat16))
    pw_bf = np.ascontiguousarray(proj_w.astype(ml_dtypes.bfloat16))
    in_maps = []
    for c in range(_NCORES):
        xs = xf[c * _T:(c + 1) * _T]
        m = {
            "xt": np.ascontiguousarray(xs.T),
            "xrb": np.ascontiguousarray(xs.astype(ml_dtypes.bfloat16)),
            "xr": np.ascontiguousarray(xs),
            "router_w": router_w,
            "router_b": router_b,
            "expert_w": ew_bf,
            "expert_b": expert_b,
            "proj_w": pw_bf,
            "proj_b": proj_b,
            "norm_w": norm_w,
        }
        in_maps.append(m)
    return in_maps


def kernel(x, router_w, router_b, expert_w, expert_b, proj_w, proj_b, norm_w):
    from concourse import bass_utils

    x = np.asarray(x, np.float32)
    router_w = np.asarray(router_w, np.float32)
    router_b = np.asarray(router_b, np.float32)
    expert_w = np.asarray(expert_w, np.float32)
    expert_b = np.asarray(expert_b, np.float32)
    proj_w = np.asarray(proj_w, np.float32)
    proj_b = np.asarray(proj_b, np.float32)
    norm_w = np.asarray(norm_w, np.float32)

    nc = _get_nc()
    xf = x.reshape(-1, _D)
    in_maps = _make_in_maps(xf, router_w, router_b, expert_w, expert_b,
                            proj_w, proj_b, norm_w)
    res = bass_utils.run_bass_kernel_spmd(nc, in_maps, core_ids=list(range(_NCORES)))
    outs = [res.results[c]["out"] for c in range(_NCORES)]
    return np.concatenate(outs, axis=0).reshape(_B, _S, _D).astype(np.float32)


# revision 6
# speedup vs baseline: 1.6859x; 1.4579x over previous
"""Trainium2 Bass kernel for EnhancedGatedFusion (MoE top-2 of 8), v2: sparse dispatch.

Strategy: data-parallel over tokens across 8 cores (T=1024 tokens each).
Per core:
  1. Router in true fp32 (top-2 selection is precision critical): stream
     x^T ko-slices, matmul vs router_w -> logits^T [E, T] in PSUM, add bias,
     transpose to token-major, dense top-2 softmax via max/mask trick.
  2. Build per-expert compact token lists ON DEVICE with no dynamic
     addressing: prefix-sum the routed-mask along tokens (tensor_tensor_scan),
     form one-hot G[t, slot] tiles via iota/is_equal, then tiny matmuls
     G^T @ [t, 1, gates] give, per compact slot: token idx, validity, gate.
     Indices round-trip through DRAM into the [16, n/16] int16 layout the
     SWDGE gather/scatter ops require (replicated across 8 partition groups).
  3. dma_gather(transpose=True) fetches the C=384-padded token set for each
     expert from token-major bf16 x in DRAM, landing feature-major
     [128, KO, C] -- the exact lhsT layout for the expert matmul.
  4. Expert matmul (bf16): x-compact stationary, expert_w streamed in halves;
     silu(h + b) * gate on scalar/vector engines (padding slots have gate 0).
  5. dma_scatter_add accumulates contributions into a DRAM comb buffer
     (fp32); padding slots carry exact zeros and are directed at trash rows
     past the real tokens so their adds cannot race real rows.
  6. comb is read back, converted bf16, PE-transposed, and projected
     (bf16) + bias + residual + RMSNorm as in the dense baseline.
"""

import sys

for _p in ("/opt/trn_rl_repo",):
    if _p not in sys.path:
        sys.path.insert(0, _p)

from contextlib import ExitStack

import numpy as np

import concourse.bass as bass
import concourse.mybir as mybir
import concourse.tile as tile
from concourse import bacc
from concourse.masks import make_identity

FP32 = mybir.dt.float32
FP32R = mybir.dt.float32r
BF16 = mybir.dt.bfloat16
I16 = mybir.dt.int16
I32 = mybir.dt.int32
AX = mybir.AxisListType
ALU = mybir.AluOpType
ACTF = mybir.ActivationFunctionType

EPS = 1e-6
NEG_BIG = -1e30


def _bcast_ap(ap, nparts=128):
    """Partition-broadcast view of a DRAM AP (step-0 partition dim)."""
    return bass.AP(tensor=ap.tensor, offset=ap.offset, ap=[[0, nparts], *ap.ap])


def build_moe_v2(D, E, T, C=384, trn_type="TRN2"):
    """Per-core sparse MoE program. D model dim, E experts, T tokens/core,
    C compact capacity per expert (multiple of 128 slots, >= max expert load)."""
    P = 128
    KO = D // P            # contraction k-tiles
    NTT = T // P           # token tiles
    NCT = C // P           # compact c-tiles per expert
    NIC = C // 16          # idx columns in the [16, C/16] swdge layout
    TRASH = T              # scatter row for padding slots
    TCOMB = ((T + C) + P - 1) // P * P  # comb rows incl. trash, multiple of 128
    NH = 2                 # expert_w streamed in halves
    HW = D // NH           # half width

    nc = bacc.Bacc(trn_type, target_bir_lowering=False, debug=False)

    xt = nc.dram_tensor("xt", [D, T], FP32, kind="ExternalInput").ap()
    xrb = nc.dram_tensor("xrb", [T, D], BF16, kind="ExternalInput").ap()
    xr = nc.dram_tensor("xr", [T, D], FP32, kind="ExternalInput").ap()
    router_w = nc.dram_tensor("router_w", [D, E], FP32, kind="ExternalInput").ap()
    router_b = nc.dram_tensor("router_b", [E], FP32, kind="ExternalInput").ap()
    expert_w = nc.dram_tensor("expert_w", [E, D, D], BF16, kind="ExternalInput").ap()
    expert_b = nc.dram_tensor("expert_b", [E, D], FP32, kind="ExternalInput").ap()
    proj_w = nc.dram_tensor("proj_w", [D, D], BF16, kind="ExternalInput").ap()
    proj_b = nc.dram_tensor("proj_b", [D], FP32, kind="ExternalInput").ap()
    norm_w = nc.dram_tensor("norm_w", [D], FP32, kind="ExternalInput").ap()
    out = nc.dram_tensor("out", [T, D], FP32, kind="ExternalOutput").ap()

    comb = nc.dram_tensor("comb_scratch", [TCOMB, D], FP32).ap()
    rowcol_dram = nc.dram_tensor("rowcol_scratch", [P + 8], FP32).ap()

    xt_r = xt.rearrange("(ko p) t -> p ko t", p=P)
    rw_r = router_w.rearrange("(ko p) e -> p ko e", p=P)
    pw_r = proj_w.rearrange("(ko p) c -> p ko c", p=P)
    comb_r = comb.rearrange("(n p) d -> p n d", p=P)

    with tile.TileContext(nc) as tc, ExitStack() as ctx:
        v = nc.vector
        s = nc.scalar

        singles = ctx.enter_context(tc.tile_pool(name="singles", bufs=1))
        idxp = ctx.enter_context(tc.tile_pool(name="idxp", bufs=1))

        # ---- resident small tensors ----
        rw_sb = singles.tile([P, KO, E], FP32)
        nc.sync.dma_start(out=rw_sb, in_=rw_r)
        rb_col = singles.tile([E, 1], FP32)
        nc.sync.dma_start(out=rb_col, in_=router_b[:, None])
        identity = singles.tile([P, P], FP32)
        make_identity(nc, identity)
        identity_bf = singles.tile([P, P], BF16)
        v.tensor_copy(out=identity_bf, in_=identity)
        eps_t = singles.tile([P, 1], FP32)
        v.memset(eps_t, EPS)
        iota1_i = singles.tile([P, C], I32)
        nc.gpsimd.iota(iota1_i, pattern=[[1, C]], base=1, channel_multiplier=0)
        iota1 = singles.tile([P, C], FP32)
        v.tensor_copy(out=iota1, in_=iota1_i)
        zero_big = singles.tile([P, 512], FP32)
        v.memset(zero_big, 0.0)

        # constants for the on-chip idx-wrap permutation:
        # Lsel[p, q] = (p % 16 == q % 16); mask8[p, pl] = (p // 16 == pl).
        # q%16 / q//16 come straight from multi-dim iota patterns; the
        # per-partition columns (p%16, p//16) take a tiny contiguous
        # DRAM round-trip (row written, read back partition-major).
        qmod_f = singles.tile([P, P], FP32)
        nc.gpsimd.iota(qmod_f, pattern=[[0, 8], [1, 16]], base=0,
                       channel_multiplier=0,
                       allow_small_or_imprecise_dtypes=True)
        qdiv_row = singles.tile([P, 8], FP32)
        nc.gpsimd.iota(qdiv_row, pattern=[[1, 8]], base=0,
                       channel_multiplier=0,
                       allow_small_or_imprecise_dtypes=True)
        nc.sync.dma_start(out=rowcol_dram[0:P], in_=qmod_f[0:1, :])
        nc.sync.dma_start(out=rowcol_dram[P:P + 8], in_=qdiv_row[0:1, :])
        pmod_f = singles.tile([P, 1], FP32)
        nc.sync.dma_start(out=pmod_f, in_=rowcol_dram[0:P, None])
        pdiv_f = singles.tile([P, 1], FP32)
        nc.sync.dma_start(
            out=pdiv_f,
            in_=bass.AP(tensor=rowcol_dram.tensor, offset=P,
                        ap=[[1, 8], [0, 16], [0, 1]]),
        )
        Lsel = singles.tile([P, P], FP32R)
        v.tensor_scalar(Lsel, qmod_f, pmod_f, None, op0=ALU.is_equal)
        plidx_f = singles.tile([P, 8], FP32)
        nc.gpsimd.iota(plidx_f, pattern=[[1, 8]], base=0, channel_multiplier=0,
                       allow_small_or_imprecise_dtypes=True)
        mask8 = singles.tile([P, 8], FP32)
        v.tensor_scalar(mask8, plidx_f, pdiv_f, None, op0=ALU.is_equal)

        # zero the comb accumulator (incl. trash rows)
        nzc = TCOMB // P
        for n in range(nzc):
            zeng = nc.scalar if n % 2 == 0 else nc.sync
            zeng.dma_start(
                out=comb_r[:, n, :],
                in_=bass.AP(tensor=zero_big.tensor, offset=zero_big.offset,
                            ap=[list(zero_big.ap[0]), [0, D // 512], [1, 512]]),
            )

        # per-expert compact metadata (gates + replicated idx arrays)
        gate_sb = [idxp.tile([P, NCT], FP32, name=f"gate{e}") for e in range(E)]
        idxg_rep = idxp.tile([P, E * NIC], I16, name="idxgr")
        idxs_rep = idxp.tile([P, E * NIC], I16, name="idxsr")

        # =========== phase 1: router (true fp32) + top-2 gates ===========
        with (
            tc.tile_pool(name="xstream", bufs=3) as xsp,
            tc.tile_pool(name="psr", bufs=1, space="PSUM") as psr,
            tc.tile_pool(name="pst", bufs=2, space="PSUM") as pst,
            tc.tile_pool(name="rsm", bufs=2) as rsm,
            tc.tile_pool(name="rowp", bufs=1) as rowp,
            tc.tile_pool(name="tokp", bufs=1) as tokp,
            tc.tile_pool(name="gmat", bufs=1) as gmat,
            tc.tile_pool(name="psg", bufs=2, space="PSUM") as psg,
        ):
            lgT_ps = psr.tile([E, T], FP32)
            for ko in range(KO):
                xs = xsp.tile([P, T], FP32, tag="xs", name=f"xs{ko}")
                eng = nc.sync if ko % 2 == 0 else nc.scalar
                eng.dma_start(out=xs, in_=xt_r[:, ko, :])
                for nch in range(T // 512):
                    nc.tensor.matmul(
                        lgT_ps[:, nch * 512:(nch + 1) * 512],
                        lhsT=rw_sb[:, ko, :],
                        rhs=xs[:, nch * 512:(nch + 1) * 512],
                        start=(ko == 0),
                        stop=(ko == KO - 1),
                    )
            lg_row = rowp.tile([E, T], FP32)
            v.tensor_scalar(lg_row, lgT_ps, rb_col, None, op0=ALU.add)

            def _fbc(ap, n):
                # broadcast view: append a step-0 innermost dim of size n
                return bass.AP(tensor=ap.tensor, offset=ap.offset,
                               ap=[*ap.ap, [0, n]])

            # token-major logits, all tiles batched: [P, NTT, E]
            lg_all = tokp.tile([P, NTT, E], FP32, name="lg_all")
            for tt in range(NTT):
                ps_l = pst.tile([P, E], FP32, tag="pstt")
                nc.tensor.transpose(ps_l, lg_row[:, tt * P:(tt + 1) * P],
                                    identity[:E, :E])
                v.tensor_copy(out=lg_all[:, tt, :], in_=ps_l)

            # batched top-2 softmax over the innermost (expert) axis
            m1 = tokp.tile([P, NTT], FP32, name="m1")
            v.tensor_reduce(m1, lg_all, axis=AX.X, op=ALU.max)
            mask1 = tokp.tile([P, NTT, E], FP32, name="mask1")
            v.tensor_tensor(out=mask1, in0=lg_all, in1=_fbc(m1, E), op=ALU.is_ge)
            lg2 = tokp.tile([P, NTT, E], FP32, name="lg2")
            v.scalar_tensor_tensor(out=lg2, in0=mask1, scalar=NEG_BIG,
                                   in1=lg_all, op0=ALU.mult, op1=ALU.add)
            m2 = tokp.tile([P, NTT], FP32, name="m2")
            v.tensor_reduce(m2, lg2, axis=AX.X, op=ALU.max)
            mask2 = tokp.tile([P, NTT, E], FP32, name="mask2")
            v.tensor_tensor(out=mask2, in0=lg2, in1=_fbc(m2, E), op=ALU.is_ge)
            d21 = tokp.tile([P, NTT], FP32, name="d21")
            v.tensor_tensor(out=d21, in0=m2, in1=m1, op=ALU.subtract)
            e2 = tokp.tile([P, NTT], FP32, name="e2")
            s.activation(e2, d21, ACTF.Exp)
            den = tokp.tile([P, NTT], FP32, name="den")
            v.tensor_scalar(den, e2, 1.0, None, op0=ALU.add)
            winv = tokp.tile([P, NTT], FP32, name="winv")
            v.reciprocal(winv, den)
            w2 = tokp.tile([P, NTT], FP32, name="w2")
            v.tensor_tensor(out=w2, in0=e2, in1=winv, op=ALU.mult)
            t2 = tokp.tile([P, NTT, E], FP32, name="t2")
            v.tensor_tensor(out=t2, in0=mask2, in1=_fbc(w2, E), op=ALU.mult)
            fw_all = tokp.tile([P, NTT, E], FP32, name="fw_all")
            v.tensor_tensor(out=fw_all, in0=mask1, in1=_fbc(winv, E), op=ALU.mult)
            v.tensor_tensor(out=fw_all, in0=fw_all, in1=t2, op=ALU.add)
            mask_all = tokp.tile([P, NTT, E], FP32, name="mask_all")
            v.tensor_scalar(mask_all, fw_all, 0.0, None, op0=ALU.is_gt)

            # expert-major gate rows -> routed-mask prefix sums
            fwT = rowp.tile([E, T], FP32)
            for tt in range(NTT):
                ps_t = pst.tile([E, P], FP32, tag="pstt")
                nc.tensor.transpose(ps_t, fw_all[:, tt, :], identity)
                v.tensor_copy(out=fwT[:, tt * P:(tt + 1) * P], in_=ps_t)
            mask_row = rowp.tile([E, T], FP32)
            v.tensor_scalar(mask_row, fwT, 0.0, None, op0=ALU.is_gt)
            zeros_row = rowp.tile([E, T], FP32)
            v.memset(zeros_row, 0.0)
            pos_row = rowp.tile([E, T], FP32)
            v.tensor_tensor_scan(
                out=pos_row, data0=mask_row, data1=zeros_row, initial=0.0,
                op0=ALU.add, op1=ALU.add,
            )
            pos_all = tokp.tile([P, NTT, E], FP32, name="pos_all")
            for tt in range(NTT):
                ps_p = pst.tile([P, E], FP32, tag="pstt")
                nc.tensor.transpose(ps_p, pos_row[:, tt * P:(tt + 1) * P],
                                    identity[:E, :E])
                v.tensor_copy(out=pos_all[:, tt, :], in_=ps_p)

            # matvec rhs: [tvec, ones, gates] per token tile, built batched
            rhs_all = tokp.tile([P, NTT, 2 + E], FP32R, name="rhs_all")
            tv2 = tokp.tile([P, NTT, 2], FP32, name="tv2")
            nc.gpsimd.iota(tv2[:, :, 0:1], pattern=[[P, NTT], [0, 1]],
                           base=0, channel_multiplier=1,
                           allow_small_or_imprecise_dtypes=True)
            v.memset(tv2[:, :, 1:2], 1.0)
            v.tensor_copy(out=rhs_all[:, :, 0:2], in_=tv2)
            v.tensor_copy(out=rhs_all[:, :, 2:2 + E], in_=fw_all)

            # one-hot G per expert (batched over token tiles) + matvecs into
            # a single PSUM strip; slot c of expert e gets (idx, valid, gates)
            iota_big = gmat.tile([P, NTT, C], FP32, name="iota_big")
            v.tensor_copy(
                out=iota_big,
                in_=bass.AP(tensor=iota1.tensor, offset=iota1.offset,
                            ap=[list(iota1.ap[0]), [0, NTT], [1, C]]),
            )
            mv_ps = psg.tile([P, E * NCT * (2 + E)], FP32, name="mv_ps")
            for e in range(E):
                g_all = gmat.tile([P, NTT, C], FP32R, tag="g_all",
                                  bufs=2, name=f"g{e}")
                pos_e = bass.AP(tensor=pos_all.tensor,
                                offset=pos_all.offset + e,
                                ap=[list(pos_all.ap[0]), [E, NTT], [0, C]])
                mask_e = bass.AP(tensor=mask_all.tensor,
                                 offset=mask_all.offset + e,
                                 ap=[list(mask_all.ap[0]), [E, NTT], [0, C]])
                v.tensor_tensor(out=g_all, in0=iota_big, in1=pos_e,
                                op=ALU.is_equal)
                v.tensor_tensor(out=g_all, in0=g_all, in1=mask_e, op=ALU.mult)
                for ct in range(NCT):
                    col = (e * NCT + ct) * (2 + E)
                    for tt in range(NTT):
                        nc.tensor.matmul(
                            mv_ps[:, col:col + 2 + E],
                            lhsT=g_all[:, tt, ct * P:(ct + 1) * P],
                            rhs=rhs_all[:, tt, :],
                            start=(tt == 0),
                            stop=(tt == NTT - 1),
                        )
            mv_all = gmat.tile([P, E, NCT, 2 + E], FP32, name="mv_all")
            v.tensor_copy(out=mv_all, in_=mv_ps)

            # batched idx/gate extraction (fp32 forms)
            idxg_f = gmat.tile([P, E, NCT], FP32, name="idxg_f")
            v.tensor_copy(out=idxg_f, in_=mv_all[:, :, :, 0])
            idxs_f = gmat.tile([P, E, NCT], FP32, name="idxs_f")
            v.tensor_scalar(idxs_f, mv_all[:, :, :, 1], float(-TRASH),
                            float(TRASH), op0=ALU.mult, op1=ALU.add)
            v.tensor_tensor(out=idxs_f, in0=idxs_f, in1=mv_all[:, :, :, 0],
                            op=ALU.add)
            for e in range(E):
                v.tensor_copy(out=gate_sb[e], in_=mv_all[:, e, :, 2 + e])

            # on-chip wrap into the swdge [16, C/16] layout, replicated
            # across partition groups: out[q, (e, b, pl)] =
            #   sum_p Lsel[p, q] * idx[p, (e, b)] * mask8[p, pl]
            # = idx value of slot c = 128 b + 16 pl + (q % 16).
            def emit_wrap(idx_f, rep_out, tagn):
                m = gmat.tile([P, E, NCT, 8], FP32R, tag="wrapm", bufs=2,
                              name=f"wrapm_{tagn}")
                v.tensor_tensor(
                    out=m,
                    in0=bass.AP(tensor=idx_f.tensor, offset=idx_f.offset,
                                ap=[*idx_f.ap, [0, 8]]),
                    in1=bass.AP(tensor=mask8.tensor, offset=mask8.offset,
                                ap=[list(mask8.ap[0]), [0, E], [0, NCT],
                                    [1, 8]]),
                    op=ALU.mult,
                )
                rep_ps = psg.tile([P, E * NCT * 8], FP32, tag="reps",
                                  name=f"reps_{tagn}")
                nc.tensor.matmul(
                    rep_ps,
                    lhsT=Lsel,
                    rhs=m[:].rearrange("p e b l -> p (e b l)"),
                    start=True,
                    stop=True,
                )
                v.tensor_copy(out=rep_out, in_=rep_ps)

            emit_wrap(idxg_f, idxg_rep, "g")
            emit_wrap(idxs_f, idxs_rep, "s")

        # =========== phase 2: sparse expert compute ===========
        expert_ctx = ExitStack()
        xg_pool = expert_ctx.enter_context(tc.tile_pool(name="xg", bufs=2))
        w_pool = expert_ctx.enter_context(tc.tile_pool(name="wp", bufs=2))
        h_pool = expert_ctx.enter_context(tc.tile_pool(name="hp", bufs=2))
        eb_pool = expert_ctx.enter_context(tc.tile_pool(name="ebp", bufs=2))
        sil_pool = expert_ctx.enter_context(tc.tile_pool(name="silp", bufs=2))
        pse = tc.alloc_tile_pool(name="pse", bufs=6, space="PSUM")

        def emit_gather(e):
            xgT = xg_pool.tile([P, KO, C], BF16, tag="xgT", name=f"xgT{e}")
            nc.gpsimd.dma_gather(
                xgT[:],
                xrb,
                idxg_rep[:, e * NIC:(e + 1) * NIC],
                C,
                C,
                D,
                elem_step=D,
                transpose=True,
                queue_num=0,
            )
            return xgT

        def emit_w(e, half):
            wh = w_pool.tile([P, KO, HW], BF16, tag="wh", bufs=3,
                             name=f"wh{e}_{half}")
            weng = nc.sync if (e + half) % 2 == 0 else nc.scalar
            we_r = expert_w[e].rearrange("(ko p) c -> p ko c", p=P)
            weng.dma_start(out=wh, in_=we_r[:, :, half * HW:(half + 1) * HW])
            return wh

        def emit_eb(e):
            eb_rep = eb_pool.tile([P, D], FP32, tag="eb", name=f"eb{e}")
            nc.scalar.dma_start(out=eb_rep, in_=_bcast_ap(expert_b[e]))
            return eb_rep

        xgT_next = emit_gather(0)
        w_next = [emit_w(0, 0), emit_w(0, 1)]
        eb_next = emit_eb(0)
        for e in range(E):
            xgT, whs, eb_rep = xgT_next, w_next, eb_next
            h_silu = h_pool.tile([P, NCT, D], FP32, tag="hs", name=f"hs{e}")
            for half in range(NH):
                wh = whs[half]
                for ct in range(NCT):
                    pss = [
                        pse.tile([P, 512], FP32, tag="ps",
                                 name=f"ps{e}_{half}_{ct}_{i}")
                        for i in range(HW // 512)
                    ]
                    for ko in range(KO):
                        for i, ps in enumerate(pss):
                            nc.tensor.matmul(
                                ps,
                                lhsT=xgT[:, ko, ct * P:(ct + 1) * P],
                                rhs=wh[:, ko, i * 512:(i + 1) * 512],
                                start=(ko == 0),
                                stop=(ko == KO - 1),
                            )
                    for i, ps in enumerate(pss):
                        fo = half * HW + i * 512
                        t1 = sil_pool.tile([P, 512], FP32, tag="t1",
                                           name=f"t1{e}_{half}_{ct}_{i}")
                        v.tensor_tensor(out=t1, in0=ps, in1=eb_rep[:, fo:fo + 512],
                                        op=ALU.add)
                        sg = sil_pool.tile([P, 512], FP32, tag="sg",
                                           name=f"sg{e}_{half}_{ct}_{i}")
                        s.activation(sg, t1, ACTF.Sigmoid)
                        v.scalar_tensor_tensor(
                            out=h_silu[:, ct, fo:fo + 512],
                            in0=t1, scalar=gate_sb[e][:, ct:ct + 1], in1=sg,
                            op0=ALU.mult, op1=ALU.mult,
                        )
                if half == 0 and e + 1 < E:
                    xgT_next = emit_gather(e + 1)
                    w_next = [emit_w(e + 1, 0), emit_w(e + 1, 1)]
                    eb_next = emit_eb(e + 1)
            nc.gpsimd.dma_scatter_add(
                comb,
                h_silu[:],
                idxs_rep[:, e * NIC:(e + 1) * NIC],
                C,
                C,
                D,
                queue_num=0,
            )

        pse.release()
        expert_ctx.close()

        # =========== phase 3: projection + residual + RMSNorm ===========
        with (
            tc.tile_pool(name="p3s", bufs=1) as p3s,
            tc.tile_pool(name="cbp", bufs=2) as cbp,
            tc.tile_pool(name="ctp", bufs=1) as ctp,
            tc.tile_pool(name="yp", bufs=2) as yp,
            tc.tile_pool(name="xresp", bufs=2) as xresp,
            tc.tile_pool(name="nsm", bufs=2) as nsm,
        ):
            pw_sb = p3s.tile([P, KO, D], BF16)
            for ko in range(KO):
                nc.sync.dma_start(out=pw_sb[:, ko, :], in_=pw_r[:, ko, :])
            nw_rep = p3s.tile([P, D], FP32)
            nc.sync.dma_start(out=nw_rep, in_=_bcast_ap(norm_w))
            prb = p3s.tile([P, D], FP32)
            nc.sync.dma_start(out=prb, in_=_bcast_ap(proj_b))

            combT = ctp.tile([P, KO, T], BF16)
            with tc.tile_pool(name="pstc", bufs=4, space="PSUM") as pstc:
                for tt in range(NTT):
                    cb = cbp.tile([P, D], FP32, tag="cb", name=f"cb{tt}")
                    nc.scalar.dma_start(out=cb, in_=comb[tt * P:(tt + 1) * P, :])
                    cbb = cbp.tile([P, D], BF16, tag="cbb", name=f"cbb{tt}")
                    v.tensor_copy(out=cbb, in_=cb)
                    for k4 in range(KO // 4):
                        ps_t = pstc.tile([P, 4, P], BF16, tag="pst")
                        for j in range(4):
                            ko = k4 * 4 + j
                            nc.tensor.transpose(
                                ps_t[:, j, :], cbb[:, ko * P:(ko + 1) * P],
                                identity_bf
                            )
                        v.tensor_copy(
                            out=combT[:, k4 * 4:(k4 + 1) * 4,
                                      tt * P:(tt + 1) * P],
                            in_=ps_t,
                        )

            psp = tc.alloc_tile_pool(name="psp", bufs=8, space="PSUM")
            HD = D // 2

            def emit_norm(tt, y_t):
                sq = nsm.tile([P, HD], FP32, tag="sq", bufs=1, name=f"sq{tt}")
                ssa = nsm.tile([P, 1], FP32, tag="ssa", name=f"ssa{tt}")
                ssb = nsm.tile([P, 1], FP32, tag="ssb", name=f"ssb{tt}")
                s.activation(sq, y_t[:, :HD], ACTF.Square, accum_out=ssa)
                s.activation(sq, y_t[:, HD:], ACTF.Square, accum_out=ssb)
                ssum = nsm.tile([P, 1], FP32, tag="ssum", name=f"ssum{tt}")
                v.tensor_tensor(out=ssum, in0=ssa, in1=ssb, op=ALU.add)
                rms = nsm.tile([P, 1], FP32, tag="rms", name=f"rms{tt}")
                s.activation(rms, ssum, ACTF.Sqrt, bias=eps_t, scale=1.0 / D)
                rinv = nsm.tile([P, 1], FP32, tag="rinv", name=f"rinv{tt}")
                v.reciprocal(rinv, rms)
                s.mul(y_t, y_t, rinv)
                v.tensor_tensor(out=y_t, in0=y_t, in1=nw_rep, op=ALU.mult)
                oeng = nc.sync if tt % 2 == 0 else nc.scalar
                oeng.dma_start(out=out[tt * P:(tt + 1) * P, :], in_=y_t)

            for tt in range(NTT):
                pso = [
                    psp.tile([P, 512], FP32, tag="pso", name=f"pso{tt}_{i}")
                    for i in range(D // 512)
                ]
                for ko in range(KO):
                    for i, ps in enumerate(pso):
                        nc.tensor.matmul(
                            ps,
                            lhsT=combT[:, ko, tt * P:(tt + 1) * P],
                            rhs=pw_sb[:, ko, i * 512:(i + 1) * 512],
                            start=(ko == 0),
                            stop=(ko == KO - 1),
                        )
                xres = xresp.tile([P, D], FP32, tag="xres", name=f"xres{tt}")
                nc.scalar.dma_start(out=xres, in_=xr[tt * P:(tt + 1) * P, :])
                y_t = yp.tile([P, D], FP32, tag="y", name=f"y{tt}")
                for i, ps in enumerate(pso):
                    y_sl = y_t[:, i * 512:(i + 1) * 512]
                    v.tensor_tensor(out=y_sl, in0=ps, in1=prb[:, i * 512:(i + 1) * 512],
                                    op=ALU.add)
                    v.tensor_tensor(out=y_sl, in0=y_sl,
                                    in1=xres[:, i * 512:(i + 1) * 512], op=ALU.add)
                emit_norm(tt, y_t)
            psp.release()

    nc.compile()
    return nc


# ---- full-problem entry point ----
_B, _S, _D, _E = 4, 2048, 2048, 8
_NCORES = 8
_T = _B * _S // _NCORES
_C = 384

_nc_cache = None


def _get_nc():
    global _nc_cache
    if _nc_cache is None:
        _nc_cache = build_moe_v2(_D, _E, _T, C=_C)
    return _nc_cache


def _make_in_maps(xf, router_w, router_b, expert_w, expert_b, proj_w, proj_b,
                  norm_w):
    import ml_dtypes
    ew_bf = np.ascontiguousarray(expert_w.astype(ml_dtypes.bfloat16))
    pw_bf = np.ascontiguousarray(proj_w.astype(ml_dtypes.bfloat16))
    in_maps = []
    for c in range(_NCORES):
        xs = xf[c * _T:(c + 1) * _T]
        m = {
            "xt": np.ascontiguousarray(xs.T),
            "xrb": np.ascontiguousarray(xs.astype(ml_dtypes.bfloat16)),
            "xr": np.ascontiguousarray(xs),
            "router_w": router_w,
            "router_b": router_b,
            "expert_w": ew_bf,
            "expert_b": expert_b,
            "proj_w": pw_bf,
            "proj_b": proj_b,
            "norm_w": norm_w,
        }
        in_maps.append(m)
    return in_maps


def kernel(x, router_w, router_b, expert_w, expert_b, proj_w, proj_b, norm_w):
    from concourse import bass_utils

    x = np.asarray(x, np.float32)
    router_w = np.asarray(router_w, np.float32)
    router_b = np.asarray(router_b, np.float32)
    expert_w = np.asarray(expert_w, np.float32)
    expert_b = np.asarray(expert_b, np.float32)
    proj_w = np.asarray(proj_w, np.float32)
    proj_b = np.asarray(proj_b, np.float32)
    norm_w = np.asarray(norm_w, np.float32)

    nc = _get_nc()
    xf = x.reshape(-1, _D)
    in_maps = _make_in_maps(xf, router_w, router_b, expert_w, expert_b,
                            proj_w, proj_b, norm_w)
    res = bass_utils.run_bass_kernel_spmd(nc, in_maps, core_ids=list(range(_NCORES)))
    outs = [res.results[c]["out"] for c in range(_NCORES)]
    return np.concatenate(outs, axis=0).reshape(_B, _S, _D).astype(np.float32)


# revision 7
# speedup vs baseline: 1.7294x; 1.0258x over previous
"""Trainium2 Bass kernel for EnhancedGatedFusion (MoE top-2 of 8), v2: sparse dispatch.

Strategy: data-parallel over tokens across 8 cores (T=1024 tokens each).
Per core:
  1. Router in true fp32 (top-2 selection is precision critical): stream
     x^T ko-slices, matmul vs router_w -> logits^T [E, T] in PSUM, add bias,
     transpose to token-major, dense top-2 softmax via max/mask trick.
  2. Build per-expert compact token lists ON DEVICE with no dynamic
     addressing: prefix-sum the routed-mask along tokens (tensor_tensor_scan),
     form one-hot G[t, slot] tiles via iota/is_equal, then tiny matmuls
     G^T @ [t, 1, gates] give, per compact slot: token idx, validity, gate.
     Indices round-trip through DRAM into the [16, n/16] int16 layout the
     SWDGE gather/scatter ops require (replicated across 8 partition groups).
  3. dma_gather(transpose=True) fetches the C=384-padded token set for each
     expert from token-major bf16 x in DRAM, landing feature-major
     [128, KO, C] -- the exact lhsT layout for the expert matmul.
  4. Expert matmul (bf16): x-compact stationary, expert_w streamed in halves;
     silu(h + b) * gate on scalar/vector engines (padding slots have gate 0).
  5. dma_scatter_add accumulates contributions into a DRAM comb buffer
     (fp32); padding slots carry exact zeros and are directed at trash rows
     past the real tokens so their adds cannot race real rows.
  6. comb is read back, converted bf16, PE-transposed, and projected
     (bf16) + bias + residual + RMSNorm as in the dense baseline.
"""

import sys

for _p in ("/opt/trn_rl_repo",):
    if _p not in sys.path:
        sys.path.insert(0, _p)

from contextlib import ExitStack

import numpy as np

import concourse.bass as bass
import concourse.mybir as mybir
import concourse.tile as tile
from concourse import bacc
from concourse.masks import make_identity

FP32 = mybir.dt.float32
FP32R = mybir.dt.float32r
BF16 = mybir.dt.bfloat16
I16 = mybir.dt.int16
I32 = mybir.dt.int32
AX = mybir.AxisListType
ALU = mybir.AluOpType
ACTF = mybir.ActivationFunctionType

EPS = 1e-6
NEG_BIG = -1e30


def _bcast_ap(ap, nparts=128):
    """Partition-broadcast view of a DRAM AP (step-0 partition dim)."""
    return bass.AP(tensor=ap.tensor, offset=ap.offset, ap=[[0, nparts], *ap.ap])


def build_moe_v2(D, E, T, C=384, trn_type="TRN2"):
    """Per-core sparse MoE program. D model dim, E experts, T tokens/core,
    C compact capacity per expert (multiple of 128 slots, >= max expert load)."""
    P = 128
    KO = D // P            # contraction k-tiles
    NTT = T // P           # token tiles
    NCT = C // P           # compact c-tiles per expert
    NIC = C // 16          # idx columns in the [16, C/16] swdge layout
    TRASH = T              # scatter row for padding slots
    TCOMB = ((T + C) + P - 1) // P * P  # comb rows incl. trash, multiple of 128
    NH = 2                 # expert_w streamed in halves
    HW = D // NH           # half width

    nc = bacc.Bacc(trn_type, target_bir_lowering=False, debug=False)

    xt = nc.dram_tensor("xt", [D, T], FP32, kind="ExternalInput").ap()
    xrb = nc.dram_tensor("xrb", [T, D], BF16, kind="ExternalInput").ap()
    xr = nc.dram_tensor("xr", [T, D], FP32, kind="ExternalInput").ap()
    router_w = nc.dram_tensor("router_w", [128, D // 128, E], FP32,
                              kind="ExternalInput").ap()
    router_b = nc.dram_tensor("router_b", [E], FP32, kind="ExternalInput").ap()
    expert_w = nc.dram_tensor("expert_w", [E, D, D], BF16, kind="ExternalInput").ap()
    expert_b = nc.dram_tensor("expert_b", [E, D], FP32, kind="ExternalInput").ap()
    proj_w = nc.dram_tensor("proj_w", [D, D], BF16, kind="ExternalInput").ap()
    proj_b = nc.dram_tensor("proj_b", [D], FP32, kind="ExternalInput").ap()
    norm_w = nc.dram_tensor("norm_w", [D], FP32, kind="ExternalInput").ap()
    out = nc.dram_tensor("out", [T, D], FP32, kind="ExternalOutput").ap()

    comb = nc.dram_tensor("comb_scratch", [TCOMB, D], FP32).ap()
    rowcol_dram = nc.dram_tensor("rowcol_scratch", [P + 8], FP32).ap()

    xt_r = xt.rearrange("(ko p) t -> p ko t", p=P)
    rw_r = router_w
    pw_r = proj_w.rearrange("(ko p) c -> p ko c", p=P)
    comb_r = comb.rearrange("(n p) d -> p n d", p=P)

    with tile.TileContext(nc) as tc, ExitStack() as ctx:
        v = nc.vector
        s = nc.scalar

        singles = ctx.enter_context(tc.tile_pool(name="singles", bufs=1))
        idxp = ctx.enter_context(tc.tile_pool(name="idxp", bufs=1))
        xg_pool = ctx.enter_context(tc.tile_pool(name="xg", bufs=2))

        # ---- resident small tensors ----
        rw_sb = singles.tile([P, KO, E], FP32)
        nc.sync.dma_start(out=rw_sb, in_=rw_r)
        rb_col = singles.tile([E, 1], FP32)
        nc.sync.dma_start(out=rb_col, in_=router_b[:, None])
        identity = singles.tile([P, P], FP32)
        make_identity(nc, identity)
        identity_bf = singles.tile([P, P], BF16)
        v.tensor_copy(out=identity_bf, in_=identity)
        eps_t = singles.tile([P, 1], FP32)
        v.memset(eps_t, EPS)
        iota1_i = singles.tile([P, C], I32)
        nc.gpsimd.iota(iota1_i, pattern=[[1, C]], base=1, channel_multiplier=0)
        iota1 = singles.tile([P, C], FP32)
        v.tensor_copy(out=iota1, in_=iota1_i)
        zero_big = singles.tile([P, 512], FP32)
        v.memset(zero_big, 0.0)

        # constants for the on-chip idx-wrap permutation:
        # Lsel[p, q] = (p % 16 == q % 16); mask8[p, pl] = (p // 16 == pl).
        # q%16 / q//16 come straight from multi-dim iota patterns; the
        # per-partition columns (p%16, p//16) take a tiny contiguous
        # DRAM round-trip (row written, read back partition-major).
        qmod_f = singles.tile([P, P], FP32)
        nc.gpsimd.iota(qmod_f, pattern=[[0, 8], [1, 16]], base=0,
                       channel_multiplier=0,
                       allow_small_or_imprecise_dtypes=True)
        qdiv_row = singles.tile([P, 8], FP32)
        nc.gpsimd.iota(qdiv_row, pattern=[[1, 8]], base=0,
                       channel_multiplier=0,
                       allow_small_or_imprecise_dtypes=True)
        nc.sync.dma_start(out=rowcol_dram[0:P], in_=qmod_f[0:1, :])
        nc.sync.dma_start(out=rowcol_dram[P:P + 8], in_=qdiv_row[0:1, :])
        pmod_f = singles.tile([P, 1], FP32)
        nc.sync.dma_start(out=pmod_f, in_=rowcol_dram[0:P, None])
        pdiv_f = singles.tile([P, 1], FP32)
        nc.sync.dma_start(
            out=pdiv_f,
            in_=bass.AP(tensor=rowcol_dram.tensor, offset=P,
                        ap=[[1, 8], [0, 16], [0, 1]]),
        )
        Lsel = singles.tile([P, P], FP32R)
        v.tensor_scalar(Lsel, qmod_f, pmod_f, None, op0=ALU.is_equal)
        plidx_f = singles.tile([P, 8], FP32)
        nc.gpsimd.iota(plidx_f, pattern=[[1, 8]], base=0, channel_multiplier=0,
                       allow_small_or_imprecise_dtypes=True)
        mask8 = singles.tile([P, 8], FP32)
        v.tensor_scalar(mask8, plidx_f, pdiv_f, None, op0=ALU.is_equal)

        # per-expert compact metadata (gates + replicated idx arrays)
        gate_sb = [idxp.tile([P, NCT], FP32, name=f"gate{e}") for e in range(E)]
        idxg_rep = idxp.tile([P, E * NIC], I16, name="idxgr")
        idxs_rep = idxp.tile([P, E * NIC], I16, name="idxsr")

        # =========== phase 1: router (true fp32) + top-2 gates ===========
        with (
            tc.tile_pool(name="xstream", bufs=3) as xsp,
            tc.tile_pool(name="psr", bufs=1, space="PSUM") as psr,
            tc.tile_pool(name="pst", bufs=2, space="PSUM") as pst,
            tc.tile_pool(name="rsm", bufs=2) as rsm,
            tc.tile_pool(name="rowp", bufs=1) as rowp,
            tc.tile_pool(name="tokp", bufs=1) as tokp,
            tc.tile_pool(name="gmat", bufs=1) as gmat,
            tc.tile_pool(name="psg", bufs=2, space="PSUM") as psg,
        ):
            lgT_ps = psr.tile([E, T], FP32)
            for ko in range(KO):
                xs = xsp.tile([P, T], FP32, tag="xs", name=f"xs{ko}")
                eng = nc.sync if ko % 2 == 0 else nc.scalar
                eng.dma_start(out=xs, in_=xt_r[:, ko, :])
                for nch in range(T // 512):
                    nc.tensor.matmul(
                        lgT_ps[:, nch * 512:(nch + 1) * 512],
                        lhsT=rw_sb[:, ko, :],
                        rhs=xs[:, nch * 512:(nch + 1) * 512],
                        start=(ko == 0),
                        stop=(ko == KO - 1),
                    )
            # zero the comb accumulator (incl. trash rows); emitted after
            # the router stream so it doesn't delay the first matmuls
            nzc = TCOMB // P
            for n in range(nzc):
                zeng = nc.scalar if n % 2 == 0 else nc.sync
                zeng.dma_start(
                    out=comb_r[:, n, :],
                    in_=bass.AP(tensor=zero_big.tensor, offset=zero_big.offset,
                                ap=[list(zero_big.ap[0]), [0, D // 512],
                                    [1, 512]]),
                )

            lg_row = rowp.tile([E, T], FP32)
            v.tensor_scalar(lg_row, lgT_ps, rb_col, None, op0=ALU.add)

            def _fbc(ap, n):
                # broadcast view: append a step-0 innermost dim of size n
                return bass.AP(tensor=ap.tensor, offset=ap.offset,
                               ap=[*ap.ap, [0, n]])

            # token-major logits, all tiles batched: [P, NTT, E]
            lg_all = tokp.tile([P, NTT, E], FP32, name="lg_all")
            for tt in range(NTT):
                ps_l = pst.tile([P, E], FP32, tag="pstt")
                nc.tensor.transpose(ps_l, lg_row[:, tt * P:(tt + 1) * P],
                                    identity[:E, :E])
                v.tensor_copy(out=lg_all[:, tt, :], in_=ps_l)

            # batched top-2 softmax over the innermost (expert) axis
            m1 = tokp.tile([P, NTT], FP32, name="m1")
            v.tensor_reduce(m1, lg_all, axis=AX.X, op=ALU.max)
            mask1 = tokp.tile([P, NTT, E], FP32, name="mask1")
            v.tensor_tensor(out=mask1, in0=lg_all, in1=_fbc(m1, E), op=ALU.is_ge)
            lg2 = tokp.tile([P, NTT, E], FP32, name="lg2")
            v.scalar_tensor_tensor(out=lg2, in0=mask1, scalar=NEG_BIG,
                                   in1=lg_all, op0=ALU.mult, op1=ALU.add)
            m2 = tokp.tile([P, NTT], FP32, name="m2")
            v.tensor_reduce(m2, lg2, axis=AX.X, op=ALU.max)
            mask2 = tokp.tile([P, NTT, E], FP32, name="mask2")
            v.tensor_tensor(out=mask2, in0=lg2, in1=_fbc(m2, E), op=ALU.is_ge)
            d21 = tokp.tile([P, NTT], FP32, name="d21")
            v.tensor_tensor(out=d21, in0=m2, in1=m1, op=ALU.subtract)
            e2 = tokp.tile([P, NTT], FP32, name="e2")
            s.activation(e2, d21, ACTF.Exp)
            den = tokp.tile([P, NTT], FP32, name="den")
            v.tensor_scalar(den, e2, 1.0, None, op0=ALU.add)
            winv = tokp.tile([P, NTT], FP32, name="winv")
            v.reciprocal(winv, den)
            w2 = tokp.tile([P, NTT], FP32, name="w2")
            v.tensor_tensor(out=w2, in0=e2, in1=winv, op=ALU.mult)
            t2 = tokp.tile([P, NTT, E], FP32, name="t2")
            v.tensor_tensor(out=t2, in0=mask2, in1=_fbc(w2, E), op=ALU.mult)
            fw_all = tokp.tile([P, NTT, E], FP32, name="fw_all")
            v.tensor_tensor(out=fw_all, in0=mask1, in1=_fbc(winv, E), op=ALU.mult)
            v.tensor_tensor(out=fw_all, in0=fw_all, in1=t2, op=ALU.add)
            mask_all = tokp.tile([P, NTT, E], FP32, name="mask_all")
            v.tensor_scalar(mask_all, fw_all, 0.0, None, op0=ALU.is_gt)

            # expert-major gate rows -> routed-mask prefix sums
            fwT = rowp.tile([E, T], FP32)
            for tt in range(NTT):
                ps_t = pst.tile([E, P], FP32, tag="pstt")
                nc.tensor.transpose(ps_t, fw_all[:, tt, :], identity)
                v.tensor_copy(out=fwT[:, tt * P:(tt + 1) * P], in_=ps_t)
            mask_row = rowp.tile([E, T], FP32)
            v.tensor_scalar(mask_row, fwT, 0.0, None, op0=ALU.is_gt)
            zeros_row = rowp.tile([E, T], FP32)
            v.memset(zeros_row, 0.0)
            pos_row = rowp.tile([E, T], FP32)
            v.tensor_tensor_scan(
                out=pos_row, data0=mask_row, data1=zeros_row, initial=0.0,
                op0=ALU.add, op1=ALU.add,
            )
            pos_all = tokp.tile([P, NTT, E], FP32, name="pos_all")
            for tt in range(NTT):
                ps_p = pst.tile([P, E], FP32, tag="pstt")
                nc.tensor.transpose(ps_p, pos_row[:, tt * P:(tt + 1) * P],
                                    identity[:E, :E])
                v.tensor_copy(out=pos_all[:, tt, :], in_=ps_p)

            # matvec rhs: [tvec, ones, gates] per token tile, built batched
            rhs_all = tokp.tile([P, NTT, 2 + E], FP32R, name="rhs_all")
            tv2 = tokp.tile([P, NTT, 2], FP32, name="tv2")
            nc.gpsimd.iota(tv2[:, :, 0:1], pattern=[[P, NTT], [0, 1]],
                           base=0, channel_multiplier=1,
                           allow_small_or_imprecise_dtypes=True)
            v.memset(tv2[:, :, 1:2], 1.0)
            v.tensor_copy(out=rhs_all[:, :, 0:2], in_=tv2)
            v.tensor_copy(out=rhs_all[:, :, 2:2 + E], in_=fw_all)

            # one-hot G per expert (batched over token tiles) + matvecs into
            # a single PSUM strip; slot c of expert e gets (idx, valid, gates)
            iota_big = gmat.tile([P, NTT, C], FP32, name="iota_big")
            v.tensor_copy(
                out=iota_big,
                in_=bass.AP(tensor=iota1.tensor, offset=iota1.offset,
                            ap=[list(iota1.ap[0]), [0, NTT], [1, C]]),
            )
            xgT_tiles = {}

            def emit_gather(e):
                xgT = xg_pool.tile([P, KO, C], BF16, tag="xgT", name=f"xgT{e}")
                nc.gpsimd.dma_gather(
                    xgT[:],
                    xrb,
                    idxg_rep[:, e * NIC:(e + 1) * NIC],
                    C,
                    C,
                    D,
                    elem_step=D,
                    transpose=True,
                    queue_num=0,
                )
                xgT_tiles[e] = xgT

            def emit_wrap(idx_f, rep_slice, tagn):
                m = gmat.tile([P, NCT, 8], FP32R, tag="wrapm", bufs=2,
                              name=f"wrapm_{tagn}")
                v.tensor_tensor(
                    out=m,
                    in0=bass.AP(tensor=idx_f.tensor, offset=idx_f.offset,
                                ap=[*idx_f.ap, [0, 8]]),
                    in1=bass.AP(tensor=mask8.tensor, offset=mask8.offset,
                                ap=[list(mask8.ap[0]), [0, NCT], [1, 8]]),
                    op=ALU.mult,
                )
                rep_ps = psg.tile([P, NCT * 8], FP32, tag="reps",
                                  name=f"reps_{tagn}")
                nc.tensor.matmul(
                    rep_ps,
                    lhsT=Lsel,
                    rhs=m[:].rearrange("p b l -> p (b l)"),
                    start=True,
                    stop=True,
                )
                v.tensor_copy(out=rep_slice, in_=rep_ps)

            for e in range(E):
                g_all = gmat.tile([P, NTT, C], FP32R, tag="g_all",
                                  bufs=2, name=f"g{e}")
                pos_e = bass.AP(tensor=pos_all.tensor,
                                offset=pos_all.offset + e,
                                ap=[list(pos_all.ap[0]), [E, NTT], [0, C]])
                mask_e = bass.AP(tensor=mask_all.tensor,
                                 offset=mask_all.offset + e,
                                 ap=[list(mask_all.ap[0]), [E, NTT], [0, C]])
                v.tensor_tensor(out=g_all, in0=iota_big, in1=pos_e,
                                op=ALU.is_equal)
                v.tensor_tensor(out=g_all, in0=g_all, in1=mask_e, op=ALU.mult)
                mv_ps = psg.tile([P, NCT * (2 + E)], FP32, tag="psmv",
                                 name=f"mv{e}")
                for ct in range(NCT):
                    col = ct * (2 + E)
                    for tt in range(NTT):
                        nc.tensor.matmul(
                            mv_ps[:, col:col + 2 + E],
                            lhsT=g_all[:, tt, ct * P:(ct + 1) * P],
                            rhs=rhs_all[:, tt, :],
                            start=(tt == 0),
                            stop=(tt == NTT - 1),
                        )
                mv_e = bass.AP(
                    tensor=mv_ps.tensor, offset=mv_ps.offset,
                    ap=[list(mv_ps.ap[0]), [2 + E, NCT], [1, 2 + E]],
                )
                idxg_f = gmat.tile([P, NCT], FP32, tag="ixf", bufs=2,
                                   name=f"idxg_f{e}")
                v.tensor_copy(out=idxg_f, in_=mv_e[:, :, 0])
                idxs_f = gmat.tile([P, NCT], FP32, tag="isf", bufs=2,
                                   name=f"idxs_f{e}")
                v.tensor_scalar(idxs_f, mv_e[:, :, 1], float(-TRASH),
                                float(TRASH), op0=ALU.mult, op1=ALU.add)
                v.tensor_tensor(out=idxs_f, in0=idxs_f, in1=mv_e[:, :, 0],
                                op=ALU.add)
                v.tensor_copy(out=gate_sb[e], in_=mv_e[:, :, 2 + e])
                emit_wrap(idxg_f, idxg_rep[:, e * NIC:(e + 1) * NIC], f"g{e}")
                emit_wrap(idxs_f, idxs_rep[:, e * NIC:(e + 1) * NIC], f"s{e}")
                if e <= 1:
                    emit_gather(e)

        # =========== phase 2: sparse expert compute ===========
        expert_ctx = ExitStack()
        w_pool = expert_ctx.enter_context(tc.tile_pool(name="wp", bufs=2))
        h_pool = expert_ctx.enter_context(tc.tile_pool(name="hp", bufs=2))
        eb_pool = expert_ctx.enter_context(tc.tile_pool(name="ebp", bufs=2))
        sil_pool = expert_ctx.enter_context(tc.tile_pool(name="silp", bufs=2))
        pse = tc.alloc_tile_pool(name="pse", bufs=6, space="PSUM")

        def emit_w(e, half):
            wh = w_pool.tile([P, KO, HW], BF16, tag="wh", bufs=3,
                             name=f"wh{e}_{half}")
            weng = nc.sync if (e + half) % 2 == 0 else nc.scalar
            we_r = expert_w[e].rearrange("(ko p) c -> p ko c", p=P)
            weng.dma_start(out=wh, in_=we_r[:, :, half * HW:(half + 1) * HW])
            return wh

        def emit_eb(e):
            eb_rep = eb_pool.tile([P, D], FP32, tag="eb", name=f"eb{e}")
            nc.scalar.dma_start(out=eb_rep, in_=_bcast_ap(expert_b[e]))
            return eb_rep

        w_next = [emit_w(0, 0), emit_w(0, 1)]
        eb_next = emit_eb(0)
        for e in range(E):
            xgT, whs, eb_rep = xgT_tiles.pop(e), w_next, eb_next
            h_silu = h_pool.tile([P, NCT, D], FP32, tag="hs", name=f"hs{e}")
            for half in range(NH):
                wh = whs[half]
                for ct in range(NCT):
                    pss = [
                        pse.tile([P, 512], FP32, tag="ps",
                                 name=f"ps{e}_{half}_{ct}_{i}")
                        for i in range(HW // 512)
                    ]
                    for ko in range(KO):
                        for i, ps in enumerate(pss):
                            nc.tensor.matmul(
                                ps,
                                lhsT=xgT[:, ko, ct * P:(ct + 1) * P],
                                rhs=wh[:, ko, i * 512:(i + 1) * 512],
                                start=(ko == 0),
                                stop=(ko == KO - 1),
                            )
                    for i, ps in enumerate(pss):
                        fo = half * HW + i * 512
                        t1 = sil_pool.tile([P, 512], FP32, tag="t1",
                                           name=f"t1{e}_{half}_{ct}_{i}")
                        v.tensor_tensor(out=t1, in0=ps, in1=eb_rep[:, fo:fo + 512],
                                        op=ALU.add)
                        sg = sil_pool.tile([P, 512], FP32, tag="sg",
                                           name=f"sg{e}_{half}_{ct}_{i}")
                        s.activation(sg, t1, ACTF.Sigmoid)
                        v.scalar_tensor_tensor(
                            out=h_silu[:, ct, fo:fo + 512],
                            in0=t1, scalar=gate_sb[e][:, ct:ct + 1], in1=sg,
                            op0=ALU.mult, op1=ALU.mult,
                        )
                if half == 0 and e + 1 < E:
                    if e + 2 < E:
                        emit_gather(e + 2)
                    w_next = [emit_w(e + 1, 0), emit_w(e + 1, 1)]
                    eb_next = emit_eb(e + 1)
            nc.gpsimd.dma_scatter_add(
                comb,
                h_silu[:],
                idxs_rep[:, e * NIC:(e + 1) * NIC],
                C,
                C,
                D,
                queue_num=0,
            )

        pse.release()
        expert_ctx.close()

        # =========== phase 3: projection + residual + RMSNorm ===========
        with (
            tc.tile_pool(name="p3s", bufs=1) as p3s,
            tc.tile_pool(name="cbp", bufs=2) as cbp,
            tc.tile_pool(name="ctp", bufs=1) as ctp,
            tc.tile_pool(name="yp", bufs=2) as yp,
            tc.tile_pool(name="xresp", bufs=2) as xresp,
            tc.tile_pool(name="nsm", bufs=2) as nsm,
        ):
            pw_sb = p3s.tile([P, KO, D], BF16)
            for ko in range(KO):
                nc.sync.dma_start(out=pw_sb[:, ko, :], in_=pw_r[:, ko, :])
            nw_rep = p3s.tile([P, D], FP32)
            nc.sync.dma_start(out=nw_rep, in_=_bcast_ap(norm_w))
            prb = p3s.tile([P, D], FP32)
            nc.sync.dma_start(out=prb, in_=_bcast_ap(proj_b))

            combT = ctp.tile([P, KO, T], BF16)
            with tc.tile_pool(name="pstc", bufs=4, space="PSUM") as pstc:
                for tt in range(NTT):
                    cb = cbp.tile([P, D], FP32, tag="cb", name=f"cb{tt}")
                    nc.scalar.dma_start(out=cb, in_=comb[tt * P:(tt + 1) * P, :])
                    cbb = cbp.tile([P, D], BF16, tag="cbb", name=f"cbb{tt}")
                    v.tensor_copy(out=cbb, in_=cb)
                    for k4 in range(KO // 4):
                        ps_t = pstc.tile([P, 4, P], BF16, tag="pst")
                        for j in range(4):
                            ko = k4 * 4 + j
                            nc.tensor.transpose(
                                ps_t[:, j, :], cbb[:, ko * P:(ko + 1) * P],
                                identity_bf
                            )
                        v.tensor_copy(
                            out=combT[:, k4 * 4:(k4 + 1) * 4,
                                      tt * P:(tt + 1) * P],
                            in_=ps_t,
                        )

            psp = tc.alloc_tile_pool(name="psp", bufs=8, space="PSUM")
            HD = D // 2

            def emit_norm(tt, y_t):
                sq = nsm.tile([P, HD], FP32, tag="sq", bufs=1, name=f"sq{tt}")
                ssa = nsm.tile([P, 1], FP32, tag="ssa", name=f"ssa{tt}")
                ssb = nsm.tile([P, 1], FP32, tag="ssb", name=f"ssb{tt}")
                s.activation(sq, y_t[:, :HD], ACTF.Square, accum_out=ssa)
                s.activation(sq, y_t[:, HD:], ACTF.Square, accum_out=ssb)
                ssum = nsm.tile([P, 1], FP32, tag="ssum", name=f"ssum{tt}")
                v.tensor_tensor(out=ssum, in0=ssa, in1=ssb, op=ALU.add)
                rms = nsm.tile([P, 1], FP32, tag="rms", name=f"rms{tt}")
                s.activation(rms, ssum, ACTF.Sqrt, bias=eps_t, scale=1.0 / D)
                rinv = nsm.tile([P, 1], FP32, tag="rinv", name=f"rinv{tt}")
                v.reciprocal(rinv, rms)
                s.mul(y_t, y_t, rinv)
                v.tensor_tensor(out=y_t, in0=y_t, in1=nw_rep, op=ALU.mult)
                oeng = nc.sync if tt % 2 == 0 else nc.scalar
                oeng.dma_start(out=out[tt * P:(tt + 1) * P, :], in_=y_t)

            for tt in range(NTT):
                pso = [
                    psp.tile([P, 512], FP32, tag="pso", name=f"pso{tt}_{i}")
                    for i in range(D // 512)
                ]
                for ko in range(KO):
                    for i, ps in enumerate(pso):
                        nc.tensor.matmul(
                            ps,
                            lhsT=combT[:, ko, tt * P:(tt + 1) * P],
                            rhs=pw_sb[:, ko, i * 512:(i + 1) * 512],
                            start=(ko == 0),
                            stop=(ko == KO - 1),
                        )
                xres = xresp.tile([P, D], FP32, tag="xres", name=f"xres{tt}")
                nc.scalar.dma_start(out=xres, in_=xr[tt * P:(tt + 1) * P, :])
                y_t = yp.tile([P, D], FP32, tag="y", name=f"y{tt}")
                for i, ps in enumerate(pso):
                    y_sl = y_t[:, i * 512:(i + 1) * 512]
                    v.tensor_tensor(out=y_sl, in0=ps, in1=prb[:, i * 512:(i + 1) * 512],
                                    op=ALU.add)
                    v.tensor_tensor(out=y_sl, in0=y_sl,
                                    in1=xres[:, i * 512:(i + 1) * 512], op=ALU.add)
                emit_norm(tt, y_t)
            psp.release()

    nc.compile()
    return nc


# ---- full-problem entry point ----
_B, _S, _D, _E = 4, 2048, 2048, 8
_NCORES = 8
_T = _B * _S // _NCORES
_C = 384

_nc_cache = None


def _get_nc():
    global _nc_cache
    if _nc_cache is None:
        _nc_cache = build_moe_v2(_D, _E, _T, C=_C)
    return _nc_cache


def _make_in_maps(xf, router_w, router_b, expert_w, expert_b, proj_w, proj_b,
                  norm_w):
    import ml_dtypes
    ew_bf = np.ascontiguousarray(expert_w.astype(ml_dtypes.bfloat16))
    rw_r_host = np.ascontiguousarray(
        router_w.reshape(_D // 128, 128, -1).transpose(1, 0, 2))
    pw_bf = np.ascontiguousarray(proj_w.astype(ml_dtypes.bfloat16))
    in_maps = []
    for c in range(_NCORES):
        xs = xf[c * _T:(c + 1) * _T]
        m = {
            "xt": np.ascontiguousarray(xs.T),
            "xrb": np.ascontiguousarray(xs.astype(ml_dtypes.bfloat16)),
            "xr": np.ascontiguousarray(xs),
            "router_w": rw_r_host,
            "router_b": router_b,
            "expert_w": ew_bf,
            "expert_b": expert_b,
            "proj_w": pw_bf,
            "proj_b": proj_b,
            "norm_w": norm_w,
        }
        in_maps.append(m)
    return in_maps


def kernel(x, router_w, router_b, expert_w, expert_b, proj_w, proj_b, norm_w):
    from concourse import bass_utils

    x = np.asarray(x, np.float32)
    router_w = np.asarray(router_w, np.float32)
    router_b = np.asarray(router_b, np.float32)
    expert_w = np.asarray(expert_w, np.float32)
    expert_b = np.asarray(expert_b, np.float32)
    proj_w = np.asarray(proj_w, np.float32)
    proj_b = np.asarray(proj_b, np.float32)
    norm_w = np.asarray(norm_w, np.float32)

    nc = _get_nc()
    xf = x.reshape(-1, _D)
    in_maps = _make_in_maps(xf, router_w, router_b, expert_w, expert_b,
                            proj_w, proj_b, norm_w)
    res = bass_utils.run_bass_kernel_spmd(nc, in_maps, core_ids=list(range(_NCORES)))
    outs = [res.results[c]["out"] for c in range(_NCORES)]
    return np.concatenate(outs, axis=0).reshape(_B, _S, _D).astype(np.float32)
